# revision 19
# baseline (speedup 1.0000x reference)
"""Trainium2 Bass kernel for nn_AdaptiveMixedCoding (8 NeuronCores).

Sharding: data-parallel over B_img (8 images per core); caps/cap_lens/alpha
replicated; caption Gram precomputed on host and shipped (tiny BLAS work).

v3 design (per core: Bi=8 imgs, R=36 regions -> 288 rows; Bc=64 caps, W=50
words, D=1024):
  - caps pre-NORMALIZED on host -> S matmul yields cosine*|img|; additive
    word mask (-6e4) folded in as a K=1 ones-row matmul; 800-wide chunks so
    each PSUM evac covers exactly 16 captions and lands in a W=64-padded
    fp16 S16 layout [128, 64*64].
  - hard attention = is_equal(S16, rowmax) with rowmax duplicated 2x so the
    compare runs in DVE 2x_1p packed mode; fp16 keeps argmax ties rare.
  - exp on ScalarE (scale=10/|img| per row, into the padded layout); all
    per-caption reductions (rowmax/den/num/qf) run as binary trees over the
    padded layout, with the widest tree stage offloaded to the Pool engine.
  - mixed = hard + (r/den)*exp in bf16 2x ops, scaled by nc*mask into the
    padded mixed tile; one XBAR DMA transpose yields all 32 caption-pair
    blocks [128(cw), 32, 128(rows)]; 32 PE matmuls against the
    pair-block-diagonal Gram; qf = tree-reduce of mixed*u.
  - two-stage software pipeline: tile i's Gram/qf phase is emitted after
    tile i+1's softmax phase so no engine FIFO cross-blocks.
  - device outputs num and qf ([rows, 64] each); host does
    out = num/sqrt(qf), invalid-row masking, and the layout transpose.
"""
import sys
import contextlib

sys.path.insert(0, '/opt/trn_rl_repo')

import numpy as np
import ml_dtypes

from concourse import bacc, tile, mybir

F32 = mybir.dt.float32
BF16 = mybir.dt.bfloat16
FP16 = mybir.dt.float16
AF = mybir.ActivationFunctionType
OP = mybir.AluOpType
AX = mybir.AxisListType

N_CORES = 8
B, R, W, D = 64, 36, 50, 1024
BC = B
BI = B // N_CORES
ROWS = BI * R               # 288
CW = BC * W                 # 3200
WP = 64                     # padded word slot
CWP = BC * WP               # 4096
KC = D // 128               # 8 contraction chunks
NP = BC // 2                # 32 caption pairs
ROW_TILES = [(0, 108), (108, 108), (216, 72)]
NCH = 8                     # 400-wide S chunks = 8 captions each
CHW = 400
EPS = 1e-8
NEGS = -60000.0             # additive word mask; fp16-safe

_CACHE = {}


def _build(r_mix: float):
    nc = bacc.Bacc("TRN2", target_bir_lowering=False, debug=False,
                   num_devices=N_CORES)

    # caps packed as 16 contiguous pieces [kc, half] -> [128, 1600]
    caps_pk = nc.declare_dram_parameter("caps_pk", [KC, 2, 128, CW // 2],
                                        BF16, isOutput=False)
    gp_in = nc.declare_dram_parameter("gp_in", [128, NP, 128], BF16,
                                      isOutput=False)
    # imgs packed in the SBUF layout [128, KC, ROWS] (contiguous rows)
    imgs_pk = nc.declare_dram_parameter("imgs_pk", [128, KC, ROWS], BF16,
                                        isOutput=False)
    adds_row = nc.declare_dram_parameter("adds_row", [1, CW], BF16,
                                         isOutput=False)    # 0 / NEGS
    ncm_row = nc.declare_dram_parameter("ncm_row", [1, CW], BF16,
                                        isOutput=False)     # nc_w * mask01
    invni_col = nc.declare_dram_parameter("invni_col", [ROWS, 1], F32,
                                          isOutput=False)   # 10/|img row|
    out_num = nc.declare_dram_parameter("out_num", [ROWS, BC], F32,
                                        isOutput=True)
    out_qf = nc.declare_dram_parameter("out_qf", [ROWS, BC], F32,
                                       isOutput=True)

    with tile.TileContext(nc) as tc, contextlib.ExitStack() as ctx:
        const = ctx.enter_context(tc.tile_pool(name="const", bufs=1))
        big = ctx.enter_context(tc.tile_pool(name="big", bufs=1))
        work = ctx.enter_context(tc.tile_pool(name="work", bufs=2))
        small = ctx.enter_context(tc.tile_pool(name="small", bufs=2))
        tree = ctx.enter_context(tc.tile_pool(name="tree", bufs=2))
        psS = ctx.enter_context(tc.tile_pool(name="psS", bufs=2, space="PSUM"))
        psU = ctx.enter_context(tc.tile_pool(name="psU", bufs=2, space="PSUM"))
        psG = ctx.enter_context(tc.tile_pool(name="psG", bufs=1, space="PSUM"))

        ones_bf = const.tile([1, 128], BF16)
        nc.gpsimd.memset(ones_bf[:], 1.0)

        # ---- input loads ------------------------------------------------
        imgsT_sb = big.tile([128, KC, ROWS], BF16)
        nc.sync.dma_start(out=imgsT_sb[:], in_=imgs_pk[:])
        addsrow_sb = const.tile([1, CW], BF16)
        nc.gpsimd.dma_start(out=addsrow_sb[:], in_=adds_row[:])
        ncmrow_sb = const.tile([1, CW], BF16)
        nc.gpsimd.dma_start(out=ncmrow_sb[:], in_=ncm_row[:])
        Gp = big.tile([128, NP, 128], BF16)
        nc.gpsimd.dma_start(out=Gp[:], in_=gp_in[:])
        # caps: 16 contiguous pieces, alternating between the two HWDGE
        # queues; first column half fully arrives before the second
        caps_sb = big.tile([128, KC, CW], BF16)
        for h in range(2):
            for kc in range(KC):
                eng = nc.sync if (kc % 2 == 0) else nc.scalar
                eng.dma_start(
                    out=caps_sb[:, kc, h * 1600:(h + 1) * 1600],
                    in_=caps_pk[kc, h])

        # ---- broadcast nc*mask row into padded layout -------------------
        ncmask = big.tile([128, CWP], BF16)
        nc.gpsimd.memset(ncmask[:], 0.0)
        for n in range(NCH):
            bps = psG.tile([128, CHW], F32, tag="b")
            nc.tensor.matmul(bps[:], ones_bf[:],
                             ncmrow_sb[:, n * CHW:(n + 1) * CHW],
                             start=True, stop=True)
            nc.scalar.activation(
                ncmask[:, n * 8 * WP:(n + 1) * 8 * WP].rearrange(
                    "p (c w) -> p c w", w=WP)[:, :, 0:W],
                bps[:].rearrange("p (c w) -> p c w", w=W), AF.Copy)

        # persistent padded ping-pong tiles; pads written once, the loop
        # only ever touches the [64, 0:50] sub-views
        S16s, expvs, hards, mpads = [], [], [], []
        for i in range(2):
            s16 = big.tile([128, CWP], FP16, name=f"s16_{i}")
            nc.gpsimd.memset(s16[:], NEGS)
            S16s.append(s16)
            ev = big.tile([128, CWP], BF16, name=f"expv_{i}")
            nc.gpsimd.memset(ev[:], 0.0)
            expvs.append(ev)
            hd = big.tile([128, CWP], BF16, name=f"hard_{i}")
            nc.gpsimd.memset(hd[:], 0.0)
            hards.append(hd)
            mq = big.tile([128, CWP], BF16, name=f"mpad_{i}")
            nc.gpsimd.memset(mq[:], 0.0)
            mpads.append(mq)

        def vp(t, rt):        # padded [rt, 64, 50] view
            return t[:rt, :].rearrange("p (c w) -> p c w", w=WP)[:, :, 0:W]

        def vdup(t, rt):      # padded [rt, 64, 25, 2] view (packed compare)
            return t[:rt, :].rearrange("p (c w) -> p c w", w=WP)[
                :, :, 0:W].rearrange("p c (k t) -> p c k t", t=2)

        def tree_reduce(x, rt, op, nm):
            # x: padded [128, CWP] tile; returns [128, BC] f32 tile.
            # binary tree stages; pads are identity elements.
            x3 = x[:rt, :].rearrange("p (c w) -> p c w", w=WP)
            a1 = tree.tile([128, BC * 32], x.dtype, tag="a1", name=f"a1{nm}")
            a13 = a1[:rt, :].rearrange("p (c w) -> p c w", w=32)
            nc.vector.tensor_tensor(a13, x3[:, :, 0:32], x3[:, :, 32:64], op)
            a2 = tree.tile([128, BC * 16], x.dtype, tag="a2", name=f"a2{nm}")
            a23 = a2[:rt, :].rearrange("p (c w) -> p c w", w=16)
            nc.vector.tensor_tensor(a23, a13[:, :, 0:16], a13[:, :, 16:32],
                                    op)
            out = small.tile([128, BC], F32, tag=f"tr{nm}", name=f"tr{nm}")
            nc.vector.tensor_reduce(out[:rt, :], a23, axis=AX.X, op=op)
            return out

        # ---- per row-tile software pipeline -----------------------------
        state = {}

        def tile_front(ti):
            r0, rt = ROW_TILES[ti]
            mm = 128 if r0 + 128 <= ROWS else rt
            S16, expv, hard, mp = (S16s[ti % 2], expvs[ti % 2],
                                   hards[ti % 2], mpads[ti % 2])
            invni_t = small.tile([128, 1], F32, tag="invni")
            nc.gpsimd.dma_start(out=invni_t[:rt, :],
                                in_=invni_col[r0:r0 + rt, :])

            # S matmul + additive mask -> fp16 S16 (padded layout)
            for n in range(NCH):
                n0 = n * CHW
                sps = psS.tile([128, CHW], F32, tag="s")
                for kc in range(KC):
                    nc.tensor.matmul(sps[:mm, :],
                                     imgsT_sb[:, kc, r0:r0 + mm],
                                     caps_sb[:, kc, n0:n0 + CHW],
                                     start=(kc == 0), stop=False)
                nc.tensor.matmul(sps[:mm, :], ones_bf[:, :mm],
                                 addsrow_sb[:, n0:n0 + CHW],
                                 start=False, stop=True)
                nc.scalar.activation(
                    S16[:rt, n * 8 * WP:(n + 1) * 8 * WP].rearrange(
                        "p (c w) -> p c w", w=WP)[:, :, 0:W],
                    sps[:rt, :].rearrange("p (c w) -> p c w", w=W), AF.Copy)

            # rowmax tree + 2x duplication for the packed compare
            rmax = tree_reduce(S16, rt, OP.max, f"mx{ti % 2}")
            rdup = small.tile([128, 2 * BC], FP16, tag="rdup")
            nc.vector.tensor_copy(
                rdup[:rt, :].rearrange("p (c t) -> p c t", t=2),
                rmax[:rt, :, None].to_broadcast([rt, BC, 2]))

            # hard = (S16 == rowmax)   [2x_1p packed fp16 compare]
            nc.vector.tensor_tensor(
                vdup(hard, rt), vdup(S16, rt),
                rdup[:rt, :].rearrange("p (c t) -> p c t", t=2)[:, :, None, :]
                    .to_broadcast([rt, BC, W // 2, 2]),
                OP.is_equal)

            # exp on ScalarE into padded layout; den via tree
            nc.scalar.activation(vp(expv, rt), vp(S16, rt), AF.Exp,
                                 scale=invni_t[:rt, :])
            den = tree_reduce(expv, rt, OP.add, f"dn{ti % 2}")
            invden = small.tile([128, BC], F32, tag="invden")
            nc.vector.reciprocal(invden[:rt, :], den[:rt, :])
            idup = small.tile([128, 2 * BC], BF16, tag="idup")
            nc.vector.tensor_scalar_mul(
                idup[:rt, :].rearrange("p (c t) -> p c t", t=2),
                invden[:rt, :, None].to_broadcast([rt, BC, 2]), r_mix)

            # mixed = hard + (r/den)*exp, then *= nc*mask -> padded mixed
            nc.vector.tensor_tensor(
                vdup(expv, rt), vdup(expv, rt),
                idup[:rt, :].rearrange("p (c t) -> p c t", t=2)[:, :, None, :]
                    .to_broadcast([rt, BC, W // 2, 2]),
                OP.mult)
            nc.vector.tensor_tensor(vp(expv, rt), vp(expv, rt), vp(hard, rt),
                                    OP.add)
            nc.vector.tensor_tensor(vp(mp, rt), vp(expv, rt), vp(ncmask, rt),
                                    OP.mult)

            # num = sum_w mixed * S   (product into hard, tree reduce)
            nc.vector.tensor_tensor(vp(hard, rt), vp(mp, rt), vp(S16, rt),
                                    OP.mult)
            num = tree_reduce(hard, rt, OP.add, f"nm{ti % 2}")

            # all 32 pair-block transposes in one XBAR DMA
            M_T = work.tile([128, NP, 128], BF16, tag="MT")
            nc.sync.dma_start_transpose(out=M_T[:], in_=mp[:])
            state[ti] = dict(mp=mp, M_T=M_T, expv=expv, num=num)

        def tile_back(ti):
            r0, rt = ROW_TILES[ti]
            st = state.pop(ti)
            mp, M_T, expv = st["mp"], st["M_T"], st["expv"]
            # u = G * mixed per pair; 4 pairs per PSUM tile, evac to U
            U = work.tile([128, CWP], BF16, tag="U")
            for g in range(NP // 4):
                ups = psU.tile([128, 512], F32, tag="u")
                for jj in range(4):
                    j = 4 * g + jj
                    nc.tensor.matmul(ups[:, 128 * jj:128 * (jj + 1)],
                                     M_T[:, j, :], Gp[:, j, :],
                                     start=True, stop=True)
                nc.scalar.activation(U[:rt, 512 * g:512 * (g + 1)],
                                     ups[:rt, :], AF.Copy)

            # qf = sum_w mixed * u  (full width: mixed pads are zero)
            nc.vector.tensor_tensor(expv[:rt, :], mp[:rt, :], U[:rt, :],
                                    OP.mult)
            qf = tree_reduce(expv, rt, OP.add, f"qf{ti % 2}")

            nc.scalar.dma_start(out=out_num[r0:r0 + rt, :],
                                in_=st["num"][:rt, :])
            nc.scalar.dma_start(out=out_qf[r0:r0 + rt, :], in_=qf[:rt, :])

        for ti in range(len(ROW_TILES) + 1):
            if ti < len(ROW_TILES):
                tile_front(ti)
            if ti >= 1:
                tile_back(ti - 1)

    nc.finalize()
    return nc


def _get_runner(r_mix: float):
    key = round(float(r_mix), 9)
    if key not in _CACHE:
        _CACHE[key] = _build(key)
    return _CACHE[key]


def _host_prep(imgs, caps, img_lens, cap_lens):
    imgs = np.ascontiguousarray(np.asarray(imgs, dtype=np.float32))
    caps = np.ascontiguousarray(np.asarray(caps, dtype=np.float32))
    cap_lens = np.asarray(cap_lens).astype(np.int64)

    ncn = np.linalg.norm(caps, axis=-1) + EPS            # [Bc, W]
    cn = caps / ncn[..., None]
    cnb = cn.astype(ml_dtypes.bfloat16).astype(np.float32)
    # caps packed so each (kc, half) DMA piece is contiguous in DRAM
    capsT = cn.reshape(CW, D).T                          # [D, CW]
    caps_pk = np.ascontiguousarray(
        capsT.reshape(KC, 128, 2, CW // 2).transpose(0, 2, 1, 3)).astype(
        ml_dtypes.bfloat16)                              # [KC, 2, 128, 1600]
    cap_mask = (np.arange(W)[None, :] < cap_lens[:, None])  # [Bc, W]
    adds_row = np.where(cap_mask.reshape(1, CW), 0.0,
                        NEGS).astype(ml_dtypes.bfloat16)
    ncm_row = np.where(cap_mask, ncn, 0.0).reshape(1, CW).astype(
        ml_dtypes.bfloat16)
    inv_ni = (10.0 / (np.linalg.norm(imgs, axis=-1) + EPS)).astype(
        np.float32)                                      # [B, R]
    # pair-block-diagonal normalized caption Gram, computed on host:
    # gp_in[p, j, m]: G_{2j} at [0:50, j, 0:50], G_{2j+1} at [64:114, j, 64:]
    G = np.einsum('cwd,cvd->cwv', cnb, cnb)              # [Bc, W, W]
    gp = np.zeros((128, NP, 128), np.float32)
    gp[:W, :, :W] = G[0::2].transpose(1, 0, 2)
    gp[WP:WP + W, :, WP:WP + W] = G[1::2].transpose(1, 0, 2)
    gp_in = gp.astype(ml_dtypes.bfloat16)

    in_maps = []
    for core in range(N_CORES):
        sl = slice(core * BI, (core + 1) * BI)
        imT = imgs[sl].reshape(ROWS, D).T                # [D, ROWS]
        imgs_pk = np.ascontiguousarray(
            imT.reshape(KC, 128, ROWS).transpose(1, 0, 2)).astype(
            ml_dtypes.bfloat16)                          # [128, KC, ROWS]
        in_maps.append({
            "caps_pk": caps_pk,
            "gp_in": gp_in,
            "imgs_pk": imgs_pk,
            "adds_row": adds_row,
            "ncm_row": ncm_row,
            "invni_col": np.ascontiguousarray(
                inv_ni[sl].reshape(ROWS, 1)),
        })
    return in_maps


def run_on_device(inputs: dict, trace: bool = False):
    """Returns (output [64,64,36] f32, BassKernelResults)."""
    from concourse.bass_utils import run_bass_kernel_spmd
    alpha = float(np.asarray(inputs["alpha"]).reshape(-1)[0])
    a = 1.0 / (1.0 + np.exp(-alpha))
    r_mix = a / max(1.0 - a, 1e-9)
    nc = _get_runner(r_mix)
    in_maps = _host_prep(inputs["imgs"], inputs["caps"], inputs["img_lens"],
                         inputs["cap_lens"])
    r = run_bass_kernel_spmd(nc, in_maps, list(range(N_CORES)), trace=trace)
    img_lens = np.asarray(inputs["img_lens"]).astype(np.int64)
    iv = (np.arange(R)[None, :] < img_lens[:, None])     # [B, R]
    outs = []
    for c in range(N_CORES):
        num = r.results[c]["out_num"].astype(np.float32)  # [ROWS, BC]
        qf = r.results[c]["out_qf"].astype(np.float32)
        o = num / (np.sqrt(np.maximum(qf, 0.0)) + 1e-30)
        o = o.reshape(BI, R, BC).transpose(0, 2, 1)       # [BI, BC, R]
        o = np.where(iv[c * BI:(c + 1) * BI, None, :], o, -1.0)
        outs.append(o)
    return np.concatenate(outs, axis=0).astype(np.float32), r


def kernel(imgs, caps, img_lens, cap_lens, alpha):
    out, _ = run_on_device({"imgs": imgs, "caps": caps, "img_lens": img_lens,
                            "cap_lens": cap_lens, "alpha": alpha})
    return out


# revision 20
# speedup vs baseline: 1.0650x; 1.0650x over previous
"""Trainium2 Bass kernel for nn_AdaptiveMixedCoding (8 NeuronCores).

Sharding: data-parallel over B_img (8 images per core); caps/cap_lens/alpha
replicated; caption Gram precomputed on host and shipped (tiny BLAS work).

v3 design (per core: Bi=8 imgs, R=36 regions -> 288 rows; Bc=64 caps, W=50
words, D=1024):
  - caps pre-NORMALIZED on host -> S matmul yields cosine*|img|; additive
    word mask (-6e4) folded in as a K=1 ones-row matmul; 800-wide chunks so
    each PSUM evac covers exactly 16 captions and lands in a W=64-padded
    fp16 S16 layout [128, 64*64].
  - hard attention = is_equal(S16, rowmax) with rowmax duplicated 2x so the
    compare runs in DVE 2x_1p packed mode; fp16 keeps argmax ties rare.
  - exp on ScalarE (scale=10/|img| per row, into the padded layout); all
    per-caption reductions (rowmax/den/num/qf) run as binary trees over the
    padded layout, with the widest tree stage offloaded to the Pool engine.
  - mixed = hard + (r/den)*exp in bf16 2x ops, scaled by nc*mask into the
    padded mixed tile; one XBAR DMA transpose yields all 32 caption-pair
    blocks [128(cw), 32, 128(rows)]; 32 PE matmuls against the
    pair-block-diagonal Gram; qf = tree-reduce of mixed*u.
  - two-stage software pipeline: tile i's Gram/qf phase is emitted after
    tile i+1's softmax phase so no engine FIFO cross-blocks.
  - device outputs num and qf ([rows, 64] each); host does
    out = num/sqrt(qf), invalid-row masking, and the layout transpose.
"""
import sys
import contextlib

sys.path.insert(0, '/opt/trn_rl_repo')

import numpy as np
import ml_dtypes

from concourse import bacc, tile, mybir

F32 = mybir.dt.float32
BF16 = mybir.dt.bfloat16
FP16 = mybir.dt.float16
AF = mybir.ActivationFunctionType
OP = mybir.AluOpType
AX = mybir.AxisListType

N_CORES = 8
B, R, W, D = 64, 36, 50, 1024
BC = B
BI = B // N_CORES
ROWS = BI * R               # 288
CW = BC * W                 # 3200
WP = 64                     # padded word slot
CWP = BC * WP               # 4096
KC = D // 128               # 8 contraction chunks
NP = BC // 2                # 32 caption pairs
ROW_TILES = [(0, 108), (108, 108), (216, 72)]
NCH = 8                     # 400-wide S chunks = 8 captions each
CHW = 400
EPS = 1e-8
NEGS = -60000.0             # additive word mask; fp16-safe

_CACHE = {}


def _build(r_mix: float):
    nc = bacc.Bacc("TRN2", target_bir_lowering=False, debug=False,
                   num_devices=N_CORES)

    # caps packed as 16 contiguous pieces [kc, half] -> [128, 1600]
    caps_pk = nc.declare_dram_parameter("caps_pk", [KC, 2, 128, CW // 2],
                                        BF16, isOutput=False)
    gp_in = nc.declare_dram_parameter("gp_in", [128, NP, 128], BF16,
                                      isOutput=False)
    # imgs packed in the SBUF layout [128, KC, ROWS] (contiguous rows)
    imgs_pk = nc.declare_dram_parameter("imgs_pk", [128, KC, ROWS], BF16,
                                        isOutput=False)
    adds_row = nc.declare_dram_parameter("adds_row", [1, CW], BF16,
                                         isOutput=False)    # 0 / NEGS
    ncm_row = nc.declare_dram_parameter("ncm_row", [1, CW], BF16,
                                        isOutput=False)     # nc_w * mask01
    invni_col = nc.declare_dram_parameter("invni_col", [ROWS, 1], F32,
                                          isOutput=False)   # 10/|img row|
    out_num = nc.declare_dram_parameter("out_num", [ROWS, BC], F32,
                                        isOutput=True)
    out_qf = nc.declare_dram_parameter("out_qf", [ROWS, BC], F32,
                                       isOutput=True)

    with tile.TileContext(nc) as tc, contextlib.ExitStack() as ctx:
        const = ctx.enter_context(tc.tile_pool(name="const", bufs=1))
        big = ctx.enter_context(tc.tile_pool(name="big", bufs=1))
        work = ctx.enter_context(tc.tile_pool(name="work", bufs=2))
        small = ctx.enter_context(tc.tile_pool(name="small", bufs=2))
        tree = ctx.enter_context(tc.tile_pool(name="tree", bufs=2))
        psS = ctx.enter_context(tc.tile_pool(name="psS", bufs=2, space="PSUM"))
        psU = ctx.enter_context(tc.tile_pool(name="psU", bufs=2, space="PSUM"))
        psG = ctx.enter_context(tc.tile_pool(name="psG", bufs=1, space="PSUM"))

        ones_bf = const.tile([1, 128], BF16)
        nc.gpsimd.memset(ones_bf[:], 1.0)

        # ---- input loads ------------------------------------------------
        imgsT_sb = big.tile([128, KC, ROWS], BF16)
        nc.sync.dma_start(out=imgsT_sb[:], in_=imgs_pk[:])
        addsrow_sb = const.tile([1, CW], BF16)
        nc.gpsimd.dma_start(out=addsrow_sb[:], in_=adds_row[:])
        ncmrow_sb = const.tile([1, CW], BF16)
        nc.gpsimd.dma_start(out=ncmrow_sb[:], in_=ncm_row[:])
        Gp = big.tile([128, NP, 128], BF16)
        nc.gpsimd.dma_start(out=Gp[:], in_=gp_in[:])
        # caps: 16 contiguous pieces, alternating between the two HWDGE
        # queues; first column half fully arrives before the second
        caps_sb = big.tile([128, KC, CW], BF16)
        for h in range(2):
            for kc in range(KC):
                eng = nc.sync if (kc % 2 == 0) else nc.scalar
                eng.dma_start(
                    out=caps_sb[:, kc, h * 1600:(h + 1) * 1600],
                    in_=caps_pk[kc, h])

        # ---- broadcast nc*mask row into padded layout -------------------
        ncmask = big.tile([128, CWP], BF16)
        nc.vector.memset(ncmask[:], 0.0)
        for n in range(NCH):
            bps = psG.tile([128, CHW], F32, tag="b")
            nc.tensor.matmul(bps[:], ones_bf[:],
                             ncmrow_sb[:, n * CHW:(n + 1) * CHW],
                             start=True, stop=True)
            nc.scalar.activation(
                ncmask[:, n * 8 * WP:(n + 1) * 8 * WP].rearrange(
                    "p (c w) -> p c w", w=WP)[:, :, 0:W],
                bps[:].rearrange("p (c w) -> p c w", w=W), AF.Copy)

        # persistent padded ping-pong tiles; pads written once, the loop
        # only ever touches the [64, 0:50] sub-views
        S16s, expvs, hards, mpads = [], [], [], []
        for i in range(2):
            s16 = big.tile([128, CWP], FP16, name=f"s16_{i}")
            nc.vector.memset(s16[:], NEGS)
            S16s.append(s16)
            ev = big.tile([128, CWP], BF16, name=f"expv_{i}")
            nc.vector.memset(ev[:], 0.0)
            expvs.append(ev)
            hd = big.tile([128, CWP], BF16, name=f"hard_{i}")
            nc.vector.memset(hd[:], 0.0)
            hards.append(hd)
            mq = big.tile([128, CWP], BF16, name=f"mpad_{i}")
            nc.vector.memset(mq[:], 0.0)
            mpads.append(mq)

        def vp(t, rt):        # padded [rt, 64, 50] view
            return t[:rt, :].rearrange("p (c w) -> p c w", w=WP)[:, :, 0:W]

        def vdup(t, rt):      # padded [rt, 64, 25, 2] view (packed compare)
            return t[:rt, :].rearrange("p (c w) -> p c w", w=WP)[
                :, :, 0:W].rearrange("p c (k t) -> p c k t", t=2)

        def tree_reduce(x, rt, op, nm):
            # x: padded [128, CWP] tile; returns [128, BC] f32 tile.
            # binary tree stages; pads are identity elements.
            x3 = x[:rt, :].rearrange("p (c w) -> p c w", w=WP)
            a1 = tree.tile([128, BC * 32], x.dtype, tag="a1", name=f"a1{nm}")
            a13 = a1[:rt, :].rearrange("p (c w) -> p c w", w=32)
            nc.vector.tensor_tensor(a13, x3[:, :, 0:32], x3[:, :, 32:64], op)
            a2 = tree.tile([128, BC * 16], x.dtype, tag="a2", name=f"a2{nm}")
            a23 = a2[:rt, :].rearrange("p (c w) -> p c w", w=16)
            nc.vector.tensor_tensor(a23, a13[:, :, 0:16], a13[:, :, 16:32],
                                    op)
            out = small.tile([128, BC], F32, tag=f"tr{nm}", name=f"tr{nm}")
            nc.vector.tensor_reduce(out[:rt, :], a23, axis=AX.X, op=op)
            return out

        # ---- per row-tile software pipeline -----------------------------
        state = {}

        def tile_front(ti):
            r0, rt = ROW_TILES[ti]
            mm = 128 if r0 + 128 <= ROWS else rt
            S16, expv, hard, mp = (S16s[ti % 2], expvs[ti % 2],
                                   hards[ti % 2], mpads[ti % 2])
            invni_t = small.tile([128, 1], F32, tag="invni")
            nc.gpsimd.dma_start(out=invni_t[:rt, :],
                                in_=invni_col[r0:r0 + rt, :])

            # S matmul + additive mask -> fp16 S16 (padded layout)
            for n in range(NCH):
                n0 = n * CHW
                sps = psS.tile([128, CHW], F32, tag="s")
                for kc in range(KC):
                    nc.tensor.matmul(sps[:mm, :],
                                     imgsT_sb[:, kc, r0:r0 + mm],
                                     caps_sb[:, kc, n0:n0 + CHW],
                                     start=(kc == 0), stop=False)
                nc.tensor.matmul(sps[:mm, :], ones_bf[:, :mm],
                                 addsrow_sb[:, n0:n0 + CHW],
                                 start=False, stop=True)
                nc.scalar.activation(
                    S16[:rt, n * 8 * WP:(n + 1) * 8 * WP].rearrange(
                        "p (c w) -> p c w", w=WP)[:, :, 0:W],
                    sps[:rt, :].rearrange("p (c w) -> p c w", w=W), AF.Copy)

            # rowmax tree + 2x duplication for the packed compare
            rmax = tree_reduce(S16, rt, OP.max, f"mx{ti % 2}")
            rdup = small.tile([128, 2 * BC], FP16, tag="rdup")
            nc.vector.tensor_copy(
                rdup[:rt, :].rearrange("p (c t) -> p c t", t=2),
                rmax[:rt, :, None].to_broadcast([rt, BC, 2]))

            # hard = (S16 == rowmax)   [2x_1p packed fp16 compare]
            nc.vector.tensor_tensor(
                vdup(hard, rt), vdup(S16, rt),
                rdup[:rt, :].rearrange("p (c t) -> p c t", t=2)[:, :, None, :]
                    .to_broadcast([rt, BC, W // 2, 2]),
                OP.is_equal)

            # exp on ScalarE into padded layout; den via tree
            nc.scalar.activation(vp(expv, rt), vp(S16, rt), AF.Exp,
                                 scale=invni_t[:rt, :])
            den = tree_reduce(expv, rt, OP.add, f"dn{ti % 2}")
            invden = small.tile([128, BC], F32, tag="invden")
            nc.vector.reciprocal(invden[:rt, :], den[:rt, :])
            idup = small.tile([128, 2 * BC], BF16, tag="idup")
            nc.vector.tensor_scalar_mul(
                idup[:rt, :].rearrange("p (c t) -> p c t", t=2),
                invden[:rt, :, None].to_broadcast([rt, BC, 2]), r_mix)

            # mixed = hard + (r/den)*exp, then *= nc*mask -> padded mixed
            nc.vector.tensor_tensor(
                vdup(expv, rt), vdup(expv, rt),
                idup[:rt, :].rearrange("p (c t) -> p c t", t=2)[:, :, None, :]
                    .to_broadcast([rt, BC, W // 2, 2]),
                OP.mult)
            nc.vector.tensor_tensor(vp(expv, rt), vp(expv, rt), vp(hard, rt),
                                    OP.add)
            nc.vector.tensor_tensor(vp(mp, rt), vp(expv, rt), vp(ncmask, rt),
                                    OP.mult)

            # num = sum_w mixed * S   (product into hard, tree reduce)
            nc.vector.tensor_tensor(vp(hard, rt), vp(mp, rt), vp(S16, rt),
                                    OP.mult)
            num = tree_reduce(hard, rt, OP.add, f"nm{ti % 2}")

            # all 32 pair-block transposes in one XBAR DMA
            M_T = work.tile([128, NP, 128], BF16, tag="MT")
            nc.sync.dma_start_transpose(out=M_T[:], in_=mp[:])
            state[ti] = dict(mp=mp, M_T=M_T, expv=expv, num=num)

        def tile_back(ti):
            r0, rt = ROW_TILES[ti]
            st = state.pop(ti)
            mp, M_T, expv = st["mp"], st["M_T"], st["expv"]
            # u = G * mixed per pair; 4 pairs per PSUM tile, evac to U
            U = work.tile([128, CWP], BF16, tag="U")
            for g in range(NP // 4):
                ups = psU.tile([128, 512], F32, tag="u")
                for jj in range(4):
                    j = 4 * g + jj
                    nc.tensor.matmul(ups[:, 128 * jj:128 * (jj + 1)],
                                     M_T[:, j, :], Gp[:, j, :],
                                     start=True, stop=True)
                nc.scalar.activation(U[:rt, 512 * g:512 * (g + 1)],
                                     ups[:rt, :], AF.Copy)

            # qf = sum_w mixed * u  (full width: mixed pads are zero)
            nc.vector.tensor_tensor(expv[:rt, :], mp[:rt, :], U[:rt, :],
                                    OP.mult)
            qf = tree_reduce(expv, rt, OP.add, f"qf{ti % 2}")

            nc.scalar.dma_start(out=out_num[r0:r0 + rt, :],
                                in_=st["num"][:rt, :])
            nc.scalar.dma_start(out=out_qf[r0:r0 + rt, :], in_=qf[:rt, :])

        for ti in range(len(ROW_TILES) + 1):
            if ti < len(ROW_TILES):
                tile_front(ti)
            if ti >= 1:
                tile_back(ti - 1)

    nc.finalize()
    return nc


def _get_runner(r_mix: float):
    key = round(float(r_mix), 9)
    if key not in _CACHE:
        _CACHE[key] = _build(key)
    return _CACHE[key]


def _host_prep(imgs, caps, img_lens, cap_lens):
    imgs = np.ascontiguousarray(np.asarray(imgs, dtype=np.float32))
    caps = np.ascontiguousarray(np.asarray(caps, dtype=np.float32))
    cap_lens = np.asarray(cap_lens).astype(np.int64)

    ncn = np.linalg.norm(caps, axis=-1) + EPS            # [Bc, W]
    cn = caps / ncn[..., None]
    cnb = cn.astype(ml_dtypes.bfloat16).astype(np.float32)
    # caps packed so each (kc, half) DMA piece is contiguous in DRAM
    capsT = cn.reshape(CW, D).T                          # [D, CW]
    caps_pk = np.ascontiguousarray(
        capsT.reshape(KC, 128, 2, CW // 2).transpose(0, 2, 1, 3)).astype(
        ml_dtypes.bfloat16)                              # [KC, 2, 128, 1600]
    cap_mask = (np.arange(W)[None, :] < cap_lens[:, None])  # [Bc, W]
    adds_row = np.where(cap_mask.reshape(1, CW), 0.0,
                        NEGS).astype(ml_dtypes.bfloat16)
    ncm_row = np.where(cap_mask, ncn, 0.0).reshape(1, CW).astype(
        ml_dtypes.bfloat16)
    inv_ni = (10.0 / (np.linalg.norm(imgs, axis=-1) + EPS)).astype(
        np.float32)                                      # [B, R]
    # pair-block-diagonal normalized caption Gram, computed on host:
    # gp_in[p, j, m]: G_{2j} at [0:50, j, 0:50], G_{2j+1} at [64:114, j, 64:]
    G = np.einsum('cwd,cvd->cwv', cnb, cnb)              # [Bc, W, W]
    gp = np.zeros((128, NP, 128), np.float32)
    gp[:W, :, :W] = G[0::2].transpose(1, 0, 2)
    gp[WP:WP + W, :, WP:WP + W] = G[1::2].transpose(1, 0, 2)
    gp_in = gp.astype(ml_dtypes.bfloat16)

    in_maps = []
    for core in range(N_CORES):
        sl = slice(core * BI, (core + 1) * BI)
        imT = imgs[sl].reshape(ROWS, D).T                # [D, ROWS]
        imgs_pk = np.ascontiguousarray(
            imT.reshape(KC, 128, ROWS).transpose(1, 0, 2)).astype(
            ml_dtypes.bfloat16)                          # [128, KC, ROWS]
        in_maps.append({
            "caps_pk": caps_pk,
            "gp_in": gp_in,
            "imgs_pk": imgs_pk,
            "adds_row": adds_row,
            "ncm_row": ncm_row,
            "invni_col": np.ascontiguousarray(
                inv_ni[sl].reshape(ROWS, 1)),
        })
    return in_maps


def run_on_device(inputs: dict, trace: bool = False):
    """Returns (output [64,64,36] f32, BassKernelResults)."""
    from concourse.bass_utils import run_bass_kernel_spmd
    alpha = float(np.asarray(inputs["alpha"]).reshape(-1)[0])
    a = 1.0 / (1.0 + np.exp(-alpha))
    r_mix = a / max(1.0 - a, 1e-9)
    nc = _get_runner(r_mix)
    in_maps = _host_prep(inputs["imgs"], inputs["caps"], inputs["img_lens"],
                         inputs["cap_lens"])
    r = run_bass_kernel_spmd(nc, in_maps, list(range(N_CORES)), trace=trace)
    img_lens = np.asarray(inputs["img_lens"]).astype(np.int64)
    iv = (np.arange(R)[None, :] < img_lens[:, None])     # [B, R]
    outs = []
    for c in range(N_CORES):
        num = r.results[c]["out_num"].astype(np.float32)  # [ROWS, BC]
        qf = r.results[c]["out_qf"].astype(np.float32)
        o = num / (np.sqrt(np.maximum(qf, 0.0)) + 1e-30)
        o = o.reshape(BI, R, BC).transpose(0, 2, 1)       # [BI, BC, R]
        o = np.where(iv[c * BI:(c + 1) * BI, None, :], o, -1.0)
        outs.append(o)
    return np.concatenate(outs, axis=0).astype(np.float32), r


def kernel(imgs, caps, img_lens, cap_lens, alpha):
    out, _ = run_on_device({"imgs": imgs, "caps": caps, "img_lens": img_lens,
                            "cap_lens": cap_lens, "alpha": alpha})
    return out


# revision 21
# speedup vs baseline: 1.2595x; 1.1827x over previous
"""Trainium2 Bass kernel for nn_AdaptiveMixedCoding (8 NeuronCores).

Sharding: data-parallel over B_img (8 images per core); caps/cap_lens/alpha
replicated; caption Gram precomputed on host and shipped (tiny BLAS work).

v3 design (per core: Bi=8 imgs, R=36 regions -> 288 rows; Bc=64 caps, W=50
words, D=1024):
  - caps pre-NORMALIZED on host -> S matmul yields cosine*|img|; additive
    word mask (-6e4) folded in as a K=1 ones-row matmul; 800-wide chunks so
    each PSUM evac covers exactly 16 captions and lands in a W=64-padded
    fp16 S16 layout [128, 64*64].
  - hard attention = is_equal(S16, rowmax) with rowmax duplicated 2x so the
    compare runs in DVE 2x_1p packed mode; fp16 keeps argmax ties rare.
  - exp on ScalarE (scale=10/|img| per row, into the padded layout); all
    per-caption reductions (rowmax/den/num/qf) run as binary trees over the
    padded layout, with the widest tree stage offloaded to the Pool engine.
  - mixed = hard + (r/den)*exp in bf16 2x ops, scaled by nc*mask into the
    padded mixed tile; one XBAR DMA transpose yields all 32 caption-pair
    blocks [128(cw), 32, 128(rows)]; 32 PE matmuls against the
    pair-block-diagonal Gram; qf = tree-reduce of mixed*u.
  - two-stage software pipeline: tile i's Gram/qf phase is emitted after
    tile i+1's softmax phase so no engine FIFO cross-blocks.
  - device outputs num and qf ([rows, 64] each); host does
    out = num/sqrt(qf), invalid-row masking, and the layout transpose.
"""
import sys
import contextlib

sys.path.insert(0, '/opt/trn_rl_repo')

import numpy as np
import ml_dtypes

from concourse import bacc, tile, mybir

F32 = mybir.dt.float32
BF16 = mybir.dt.bfloat16
FP16 = mybir.dt.float16
AF = mybir.ActivationFunctionType
OP = mybir.AluOpType
AX = mybir.AxisListType

N_CORES = 8
B, R, W, D = 64, 36, 50, 1024
BC = B
BI = B // N_CORES
ROWS = BI * R               # 288
CW = BC * W                 # 3200
WP = 64                     # padded word slot
CWP = BC * WP               # 4096
KC = D // 128               # 8 contraction chunks
NP = BC // 2                # 32 caption pairs
ROW_TILES = [(0, 108), (108, 108), (216, 72)]
NCH = 8                     # 400-wide S chunks = 8 captions each
CHW = 400
EPS = 1e-8
NEGS = -60000.0             # additive word mask; fp16-safe

_CACHE = {}


def _build(r_mix: float):
    nc = bacc.Bacc("TRN2", target_bir_lowering=False, debug=False,
                   num_devices=N_CORES)

    # caps packed as 16 contiguous pieces [kc, half] -> [128, 1600]
    caps_pk = nc.declare_dram_parameter("caps_pk", [KC, 2, 128, CW // 2],
                                        BF16, isOutput=False)
    gp_in = nc.declare_dram_parameter("gp_in", [128, NP, 128], BF16,
                                      isOutput=False)
    # imgs packed in the SBUF layout [128, KC, ROWS] (contiguous rows)
    imgs_pk = nc.declare_dram_parameter("imgs_pk", [128, KC, ROWS], BF16,
                                        isOutput=False)
    adds_row = nc.declare_dram_parameter("adds_row", [1, CW], BF16,
                                         isOutput=False)    # 0 / NEGS
    ncm_row = nc.declare_dram_parameter("ncm_row", [1, CW], BF16,
                                        isOutput=False)     # nc_w * mask01
    invni_col = nc.declare_dram_parameter("invni_col", [ROWS, 1], F32,
                                          isOutput=False)   # 10/|img row|
    out_num = nc.declare_dram_parameter("out_num", [ROWS, BC], F32,
                                        isOutput=True)
    out_qf = nc.declare_dram_parameter("out_qf", [ROWS, BC], F32,
                                       isOutput=True)

    with tile.TileContext(nc) as tc, contextlib.ExitStack() as ctx:
        const = ctx.enter_context(tc.tile_pool(name="const", bufs=1))
        big = ctx.enter_context(tc.tile_pool(name="big", bufs=1))
        work = ctx.enter_context(tc.tile_pool(name="work", bufs=2))
        small = ctx.enter_context(tc.tile_pool(name="small", bufs=2))
        tree = ctx.enter_context(tc.tile_pool(name="tree", bufs=2))
        psS = ctx.enter_context(tc.tile_pool(name="psS", bufs=2, space="PSUM"))
        psU = ctx.enter_context(tc.tile_pool(name="psU", bufs=2, space="PSUM"))
        psG = ctx.enter_context(tc.tile_pool(name="psG", bufs=1, space="PSUM"))

        ones_bf = const.tile([1, 128], BF16)
        nc.gpsimd.memset(ones_bf[:], 1.0)

        # ---- input loads ------------------------------------------------
        imgsT_sb = big.tile([128, KC, ROWS], BF16)
        nc.sync.dma_start(out=imgsT_sb[:], in_=imgs_pk[:])
        addsrow_sb = const.tile([1, CW], BF16)
        nc.gpsimd.dma_start(out=addsrow_sb[:], in_=adds_row[:])
        ncmrow_sb = const.tile([1, CW], BF16)
        nc.gpsimd.dma_start(out=ncmrow_sb[:], in_=ncm_row[:])
        Gp = big.tile([128, NP, 128], BF16)
        nc.gpsimd.dma_start(out=Gp[:], in_=gp_in[:])
        # caps: 16 contiguous pieces, alternating between the two HWDGE
        # queues; first column half fully arrives before the second
        caps_sb = big.tile([128, KC, CW], BF16)
        for h in range(2):
            for kc in range(KC):
                eng = nc.sync if (kc % 2 == 0) else nc.scalar
                eng.dma_start(
                    out=caps_sb[:, kc, h * 1600:(h + 1) * 1600],
                    in_=caps_pk[kc, h])

        # ---- broadcast nc*mask row into padded layout -------------------
        ncmask = big.tile([128, CWP], BF16)
        nc.vector.memset(
            ncmask.rearrange("p (c w) -> p c w", w=WP)[:, :, W:WP], 0.0)
        for n in range(NCH):
            bps = psG.tile([128, CHW], F32, tag="b")
            nc.tensor.matmul(bps[:], ones_bf[:],
                             ncmrow_sb[:, n * CHW:(n + 1) * CHW],
                             start=True, stop=True)
            nc.scalar.activation(
                ncmask[:, n * 8 * WP:(n + 1) * 8 * WP].rearrange(
                    "p (c w) -> p c w", w=WP)[:, :, 0:W],
                bps[:].rearrange("p (c w) -> p c w", w=W), AF.Copy)

        # persistent padded ping-pong tiles; pads written once, the loop
        # only ever touches the [64, 0:50] sub-views
        def pads(t):
            return t.rearrange("p (c w) -> p c w", w=WP)[:, :, W:WP]

        S16s, expvs, hards, mpads = [], [], [], []
        for i in range(2):
            s16 = big.tile([128, CWP], FP16, name=f"s16_{i}")
            nc.vector.memset(pads(s16), NEGS)
            S16s.append(s16)
            ev = big.tile([128, CWP], BF16, name=f"expv_{i}")
            nc.vector.memset(pads(ev), 0.0)
            expvs.append(ev)
            hd = big.tile([128, CWP], BF16, name=f"hard_{i}")
            nc.vector.memset(pads(hd), 0.0)
            hards.append(hd)
            # mixed pads feed the XBAR/u-matmuls: full memset
            mq = big.tile([128, CWP], BF16, name=f"mpad_{i}")
            nc.vector.memset(mq[:], 0.0)
            mpads.append(mq)

        def vp(t, rt):        # padded [rt, 64, 50] view
            return t[:rt, :].rearrange("p (c w) -> p c w", w=WP)[:, :, 0:W]

        def vdup(t, rt):      # padded [rt, 64, 25, 2] view (packed compare)
            return t[:rt, :].rearrange("p (c w) -> p c w", w=WP)[
                :, :, 0:W].rearrange("p c (k t) -> p c k t", t=2)

        def tree_reduce(x, rt, op, nm):
            # x: padded [128, CWP] tile; returns [128, BC] f32 tile.
            # binary tree stages; pads are identity elements.
            x3 = x[:rt, :].rearrange("p (c w) -> p c w", w=WP)
            a1 = tree.tile([128, BC * 32], x.dtype, tag="a1", name=f"a1{nm}")
            a13 = a1[:rt, :].rearrange("p (c w) -> p c w", w=32)
            nc.vector.tensor_tensor(a13, x3[:, :, 0:32], x3[:, :, 32:64], op)
            a2 = tree.tile([128, BC * 16], x.dtype, tag="a2", name=f"a2{nm}")
            a23 = a2[:rt, :].rearrange("p (c w) -> p c w", w=16)
            nc.vector.tensor_tensor(a23, a13[:, :, 0:16], a13[:, :, 16:32],
                                    op)
            out = small.tile([128, BC], F32, tag=f"tr{nm}", name=f"tr{nm}")
            nc.vector.tensor_reduce(out[:rt, :], a23, axis=AX.X, op=op)
            return out

        # ---- per row-tile software pipeline -----------------------------
        state = {}

        def tile_front(ti):
            r0, rt = ROW_TILES[ti]
            mm = 128 if r0 + 128 <= ROWS else rt
            S16, expv, hard, mp = (S16s[ti % 2], expvs[ti % 2],
                                   hards[ti % 2], mpads[ti % 2])
            invni_t = small.tile([128, 1], F32, tag="invni")
            nc.gpsimd.dma_start(out=invni_t[:rt, :],
                                in_=invni_col[r0:r0 + rt, :])

            # S matmul + additive mask -> fp16 S16 (padded layout)
            for n in range(NCH):
                n0 = n * CHW
                sps = psS.tile([128, CHW], F32, tag="s")
                for kc in range(KC):
                    nc.tensor.matmul(sps[:mm, :],
                                     imgsT_sb[:, kc, r0:r0 + mm],
                                     caps_sb[:, kc, n0:n0 + CHW],
                                     start=(kc == 0), stop=False)
                nc.tensor.matmul(sps[:mm, :], ones_bf[:, :mm],
                                 addsrow_sb[:, n0:n0 + CHW],
                                 start=False, stop=True)
                nc.scalar.activation(
                    S16[:rt, n * 8 * WP:(n + 1) * 8 * WP].rearrange(
                        "p (c w) -> p c w", w=WP)[:, :, 0:W],
                    sps[:rt, :].rearrange("p (c w) -> p c w", w=W), AF.Copy)

            # rowmax tree + 2x duplication for the packed compare
            rmax = tree_reduce(S16, rt, OP.max, f"mx{ti % 2}")
            rdup = small.tile([128, 2 * BC], FP16, tag="rdup")
            nc.vector.tensor_copy(
                rdup[:rt, :].rearrange("p (c t) -> p c t", t=2),
                rmax[:rt, :, None].to_broadcast([rt, BC, 2]))

            # hard = (S16 == rowmax)   [2x_1p packed fp16 compare]
            nc.vector.tensor_tensor(
                vdup(hard, rt), vdup(S16, rt),
                rdup[:rt, :].rearrange("p (c t) -> p c t", t=2)[:, :, None, :]
                    .to_broadcast([rt, BC, W // 2, 2]),
                OP.is_equal)

            # exp on ScalarE into padded layout; den via tree
            nc.scalar.activation(vp(expv, rt), vp(S16, rt), AF.Exp,
                                 scale=invni_t[:rt, :])
            den = tree_reduce(expv, rt, OP.add, f"dn{ti % 2}")
            invden = small.tile([128, BC], F32, tag="invden")
            nc.vector.reciprocal(invden[:rt, :], den[:rt, :])
            idup = small.tile([128, 2 * BC], BF16, tag="idup")
            nc.vector.tensor_scalar_mul(
                idup[:rt, :].rearrange("p (c t) -> p c t", t=2),
                invden[:rt, :, None].to_broadcast([rt, BC, 2]), r_mix)

            # mixed = hard + (r/den)*exp, then *= nc*mask -> padded mixed
            nc.vector.tensor_tensor(
                vdup(expv, rt), vdup(expv, rt),
                idup[:rt, :].rearrange("p (c t) -> p c t", t=2)[:, :, None, :]
                    .to_broadcast([rt, BC, W // 2, 2]),
                OP.mult)
            nc.vector.tensor_tensor(vp(expv, rt), vp(expv, rt), vp(hard, rt),
                                    OP.add)
            nc.vector.tensor_tensor(vp(mp, rt), vp(expv, rt), vp(ncmask, rt),
                                    OP.mult)

            # num = sum_w mixed * S   (product into hard, tree reduce)
            nc.vector.tensor_tensor(vp(hard, rt), vp(mp, rt), vp(S16, rt),
                                    OP.mult)
            num = tree_reduce(hard, rt, OP.add, f"nm{ti % 2}")

            # all 32 pair-block transposes in one XBAR DMA
            M_T = work.tile([128, NP, 128], BF16, tag="MT")
            nc.sync.dma_start_transpose(out=M_T[:], in_=mp[:])
            state[ti] = dict(mp=mp, M_T=M_T, expv=expv, num=num)

        def tile_back(ti):
            r0, rt = ROW_TILES[ti]
            st = state.pop(ti)
            mp, M_T, expv = st["mp"], st["M_T"], st["expv"]
            # u = G * mixed per pair; 4 pairs per PSUM tile, evac to U
            U = work.tile([128, CWP], BF16, tag="U")
            for g in range(NP // 4):
                ups = psU.tile([128, 512], F32, tag="u")
                for jj in range(4):
                    j = 4 * g + jj
                    nc.tensor.matmul(ups[:, 128 * jj:128 * (jj + 1)],
                                     M_T[:, j, :], Gp[:, j, :],
                                     start=True, stop=True)
                nc.scalar.activation(U[:rt, 512 * g:512 * (g + 1)],
                                     ups[:rt, :], AF.Copy)

            # qf = sum_w mixed * u  (full width: mixed pads are zero)
            nc.vector.tensor_tensor(expv[:rt, :], mp[:rt, :], U[:rt, :],
                                    OP.mult)
            qf = tree_reduce(expv, rt, OP.add, f"qf{ti % 2}")

            nc.scalar.dma_start(out=out_num[r0:r0 + rt, :],
                                in_=st["num"][:rt, :])
            nc.scalar.dma_start(out=out_qf[r0:r0 + rt, :], in_=qf[:rt, :])

        for ti in range(len(ROW_TILES) + 1):
            if ti < len(ROW_TILES):
                tile_front(ti)
            if ti >= 1:
                tile_back(ti - 1)

    nc.finalize()
    return nc


def _get_runner(r_mix: float):
    key = round(float(r_mix), 9)
    if key not in _CACHE:
        _CACHE[key] = _build(key)
    return _CACHE[key]


def _host_prep(imgs, caps, img_lens, cap_lens):
    imgs = np.ascontiguousarray(np.asarray(imgs, dtype=np.float32))
    caps = np.ascontiguousarray(np.asarray(caps, dtype=np.float32))
    cap_lens = np.asarray(cap_lens).astype(np.int64)

    ncn = np.linalg.norm(caps, axis=-1) + EPS            # [Bc, W]
    cn = caps / ncn[..., None]
    cnb = cn.astype(ml_dtypes.bfloat16).astype(np.float32)
    # caps packed so each (kc, half) DMA piece is contiguous in DRAM
    capsT = cn.reshape(CW, D).T                          # [D, CW]
    caps_pk = np.ascontiguousarray(
        capsT.reshape(KC, 128, 2, CW // 2).transpose(0, 2, 1, 3)).astype(
        ml_dtypes.bfloat16)                              # [KC, 2, 128, 1600]
    cap_mask = (np.arange(W)[None, :] < cap_lens[:, None])  # [Bc, W]
    adds_row = np.where(cap_mask.reshape(1, CW), 0.0,
                        NEGS).astype(ml_dtypes.bfloat16)
    ncm_row = np.where(cap_mask, ncn, 0.0).reshape(1, CW).astype(
        ml_dtypes.bfloat16)
    inv_ni = (10.0 / (np.linalg.norm(imgs, axis=-1) + EPS)).astype(
        np.float32)                                      # [B, R]
    # pair-block-diagonal normalized caption Gram, computed on host:
    # gp_in[p, j, m]: G_{2j} at [0:50, j, 0:50], G_{2j+1} at [64:114, j, 64:]
    G = np.einsum('cwd,cvd->cwv', cnb, cnb)              # [Bc, W, W]
    gp = np.zeros((128, NP, 128), np.float32)
    gp[:W, :, :W] = G[0::2].transpose(1, 0, 2)
    gp[WP:WP + W, :, WP:WP + W] = G[1::2].transpose(1, 0, 2)
    gp_in = gp.astype(ml_dtypes.bfloat16)

    in_maps = []
    for core in range(N_CORES):
        sl = slice(core * BI, (core + 1) * BI)
        imT = imgs[sl].reshape(ROWS, D).T                # [D, ROWS]
        imgs_pk = np.ascontiguousarray(
            imT.reshape(KC, 128, ROWS).transpose(1, 0, 2)).astype(
            ml_dtypes.bfloat16)                          # [128, KC, ROWS]
        in_maps.append({
            "caps_pk": caps_pk,
            "gp_in": gp_in,
            "imgs_pk": imgs_pk,
            "adds_row": adds_row,
            "ncm_row": ncm_row,
            "invni_col": np.ascontiguousarray(
                inv_ni[sl].reshape(ROWS, 1)),
        })
    return in_maps


def run_on_device(inputs: dict, trace: bool = False):
    """Returns (output [64,64,36] f32, BassKernelResults)."""
    from concourse.bass_utils import run_bass_kernel_spmd
    alpha = float(np.asarray(inputs["alpha"]).reshape(-1)[0])
    a = 1.0 / (1.0 + np.exp(-alpha))
    r_mix = a / max(1.0 - a, 1e-9)
    nc = _get_runner(r_mix)
    in_maps = _host_prep(inputs["imgs"], inputs["caps"], inputs["img_lens"],
                         inputs["cap_lens"])
    r = run_bass_kernel_spmd(nc, in_maps, list(range(N_CORES)), trace=trace)
    img_lens = np.asarray(inputs["img_lens"]).astype(np.int64)
    iv = (np.arange(R)[None, :] < img_lens[:, None])     # [B, R]
    outs = []
    for c in range(N_CORES):
        num = r.results[c]["out_num"].astype(np.float32)  # [ROWS, BC]
        qf = r.results[c]["out_qf"].astype(np.float32)
        o = num / (np.sqrt(np.maximum(qf, 0.0)) + 1e-30)
        o = o.reshape(BI, R, BC).transpose(0, 2, 1)       # [BI, BC, R]
        o = np.where(iv[c * BI:(c + 1) * BI, None, :], o, -1.0)
        outs.append(o)
    return np.concatenate(outs, axis=0).astype(np.float32), r


def kernel(imgs, caps, img_lens, cap_lens, alpha):
    out, _ = run_on_device({"imgs": imgs, "caps": caps, "img_lens": img_lens,
                            "cap_lens": cap_lens, "alpha": alpha})
    return out


# revision 23
# speedup vs baseline: 1.2604x; 1.0007x over previous
"""Trainium2 Bass kernel for nn_AdaptiveMixedCoding (8 NeuronCores).

Sharding: data-parallel over B_img (8 images per core); caps/cap_lens/alpha
replicated; caption Gram precomputed on host and shipped (tiny BLAS work).

v3 design (per core: Bi=8 imgs, R=36 regions -> 288 rows; Bc=64 caps, W=50
words, D=1024):
  - caps pre-NORMALIZED on host -> S matmul yields cosine*|img|; additive
    word mask (-6e4) folded in as a K=1 ones-row matmul; 800-wide chunks so
    each PSUM evac covers exactly 16 captions and lands in a W=64-padded
    fp16 S16 layout [128, 64*64].
  - hard attention = is_equal(S16, rowmax) with rowmax duplicated 2x so the
    compare runs in DVE 2x_1p packed mode; fp16 keeps argmax ties rare.
  - exp on ScalarE (scale=10/|img| per row, into the padded layout); all
    per-caption reductions (rowmax/den/num/qf) run as binary trees over the
    padded layout, with the widest tree stage offloaded to the Pool engine.
  - mixed = hard + (r/den)*exp in bf16 2x ops, scaled by nc*mask into the
    padded mixed tile; one XBAR DMA transpose yields all 32 caption-pair
    blocks [128(cw), 32, 128(rows)]; 32 PE matmuls against the
    pair-block-diagonal Gram; qf = tree-reduce of mixed*u.
  - two-stage software pipeline: tile i's Gram/qf phase is emitted after
    tile i+1's softmax phase so no engine FIFO cross-blocks.
  - device outputs num and qf ([rows, 64] each); host does
    out = num/sqrt(qf), invalid-row masking, and the layout transpose.
"""
import sys
import contextlib

sys.path.insert(0, '/opt/trn_rl_repo')

import numpy as np
import ml_dtypes

from concourse import bacc, tile, mybir

F32 = mybir.dt.float32
BF16 = mybir.dt.bfloat16
FP16 = mybir.dt.float16
AF = mybir.ActivationFunctionType
OP = mybir.AluOpType
AX = mybir.AxisListType

N_CORES = 8
B, R, W, D = 64, 36, 50, 1024
BC = B
BI = B // N_CORES
ROWS = BI * R               # 288
CW = BC * W                 # 3200
WP = 64                     # padded word slot
CWP = BC * WP               # 4096
KC = D // 128               # 8 contraction chunks
NP = BC // 2                # 32 caption pairs
ROW_TILES = [(0, 108), (108, 108), (216, 72)]
NCH = 8                     # 400-wide S chunks = 8 captions each
CHW = 400
EPS = 1e-8
NEGS = -60000.0             # additive word mask; fp16-safe

_CACHE = {}


def _build(r_mix: float):
    nc = bacc.Bacc("TRN2", target_bir_lowering=False, debug=False,
                   num_devices=N_CORES)

    # caps packed as 16 contiguous pieces [kc, half] -> [128, 1600]
    caps_pk = nc.declare_dram_parameter("caps_pk", [KC, 2, 128, CW // 2],
                                        BF16, isOutput=False)
    gp_in = nc.declare_dram_parameter("gp_in", [128, NP, 128], BF16,
                                      isOutput=False)
    # imgs packed in the SBUF layout [128, KC, ROWS] (contiguous rows)
    imgs_pk = nc.declare_dram_parameter("imgs_pk", [128, KC, ROWS], BF16,
                                        isOutput=False)
    adds_row = nc.declare_dram_parameter("adds_row", [1, CW], BF16,
                                         isOutput=False)    # 0 / NEGS
    ncm_row = nc.declare_dram_parameter("ncm_row", [1, CW], BF16,
                                        isOutput=False)     # nc_w * mask01
    invni_col = nc.declare_dram_parameter("invni_col", [ROWS, 1], F32,
                                          isOutput=False)   # 10/|img row|
    out_num = nc.declare_dram_parameter("out_num", [ROWS, BC], F32,
                                        isOutput=True)
    out_qf = nc.declare_dram_parameter("out_qf", [ROWS, BC], F32,
                                       isOutput=True)

    with tile.TileContext(nc) as tc, contextlib.ExitStack() as ctx:
        const = ctx.enter_context(tc.tile_pool(name="const", bufs=1))
        big = ctx.enter_context(tc.tile_pool(name="big", bufs=1))
        work = ctx.enter_context(tc.tile_pool(name="work", bufs=2))
        small = ctx.enter_context(tc.tile_pool(name="small", bufs=2))
        tree = ctx.enter_context(tc.tile_pool(name="tree", bufs=2))
        psS = ctx.enter_context(tc.tile_pool(name="psS", bufs=2, space="PSUM"))
        psU = ctx.enter_context(tc.tile_pool(name="psU", bufs=2, space="PSUM"))
        psG = ctx.enter_context(tc.tile_pool(name="psG", bufs=1, space="PSUM"))

        ones_bf = const.tile([1, 128], BF16)
        nc.gpsimd.memset(ones_bf[:], 1.0)

        # ---- input loads ------------------------------------------------
        imgsT_sb = big.tile([128, KC, ROWS], BF16)
        nc.sync.dma_start(out=imgsT_sb[:], in_=imgs_pk[:])
        addsrow_sb = const.tile([1, CW], BF16)
        nc.gpsimd.dma_start(out=addsrow_sb[:], in_=adds_row[:])
        ncmrow_sb = const.tile([1, CW], BF16)
        nc.gpsimd.dma_start(out=ncmrow_sb[:], in_=ncm_row[:])
        Gp = big.tile([128, NP, 128], BF16)
        nc.gpsimd.dma_start(out=Gp[:], in_=gp_in[:])
        # caps: 16 contiguous pieces, alternating between the two HWDGE
        # queues; first column half fully arrives before the second
        caps_sb = big.tile([128, KC, CW], BF16)
        for h in range(2):
            for kc in range(KC):
                eng = nc.sync if (kc % 2 == 0) else nc.scalar
                eng.dma_start(
                    out=caps_sb[:, kc, h * 1600:(h + 1) * 1600],
                    in_=caps_pk[kc, h])

        # ---- broadcast nc*mask row into padded layout -------------------
        ncmask = big.tile([128, CWP], BF16)
        nc.vector.memset(
            ncmask.rearrange("p (c w) -> p c w", w=WP)[:, :, W:WP], 0.0)
        for n in range(NCH):
            bps = psG.tile([128, CHW], F32, tag="b")
            nc.tensor.matmul(bps[:], ones_bf[:],
                             ncmrow_sb[:, n * CHW:(n + 1) * CHW],
                             start=True, stop=True)
            nc.scalar.activation(
                ncmask[:, n * 8 * WP:(n + 1) * 8 * WP].rearrange(
                    "p (c w) -> p c w", w=WP)[:, :, 0:W],
                bps[:].rearrange("p (c w) -> p c w", w=W), AF.Copy)

        # persistent padded ping-pong tiles; pads written once, the loop
        # only ever touches the [64, 0:50] sub-views
        def pads(t):
            return t.rearrange("p (c w) -> p c w", w=WP)[:, :, W:WP]

        S16s, expvs, hards, mpads = [], [], [], []
        for i in range(2):
            s16 = big.tile([128, CWP], FP16, name=f"s16_{i}")
            nc.vector.memset(pads(s16), NEGS)
            S16s.append(s16)
            ev = big.tile([128, CWP], BF16, name=f"expv_{i}")
            nc.vector.memset(pads(ev), 0.0)
            expvs.append(ev)
            hd = big.tile([128, CWP], BF16, name=f"hard_{i}")
            nc.vector.memset(pads(hd), 0.0)
            hards.append(hd)
            # mixed pads feed the XBAR/u-matmuls: full memset
            mq = big.tile([128, CWP], BF16, name=f"mpad_{i}")
            nc.vector.memset(mq[:], 0.0)
            mpads.append(mq)

        def vp(t, rt):        # padded [rt, 64, 50] view
            return t[:rt, :].rearrange("p (c w) -> p c w", w=WP)[:, :, 0:W]

        def vdup(t, rt):      # padded [rt, 64, 25, 2] view (packed compare)
            return t[:rt, :].rearrange("p (c w) -> p c w", w=WP)[
                :, :, 0:W].rearrange("p c (k t) -> p c k t", t=2)

        def tree_reduce(x, rt, op, nm):
            # x: padded [128, CWP] tile; returns [128, BC] f32 tile.
            # binary tree stages; pads are identity elements.
            x3 = x[:rt, :].rearrange("p (c w) -> p c w", w=WP)
            a1 = tree.tile([128, BC * 32], x.dtype, tag="a1", name=f"a1{nm}")
            a13 = a1[:rt, :].rearrange("p (c w) -> p c w", w=32)
            nc.vector.tensor_tensor(a13, x3[:, :, 0:32], x3[:, :, 32:64], op)
            a2 = tree.tile([128, BC * 16], x.dtype, tag="a2", name=f"a2{nm}")
            a23 = a2[:rt, :].rearrange("p (c w) -> p c w", w=16)
            nc.vector.tensor_tensor(a23, a13[:, :, 0:16], a13[:, :, 16:32],
                                    op)
            out = small.tile([128, BC], F32, tag=f"tr{nm}", name=f"tr{nm}")
            nc.vector.tensor_reduce(out[:rt, :], a23, axis=AX.X, op=op)
            return out

        # ---- per row-tile software pipeline -----------------------------
        state = {}

        def tile_front(ti):
            r0, rt = ROW_TILES[ti]
            mm = 128 if r0 + 128 <= ROWS else rt
            S16, expv, hard, mp = (S16s[ti % 2], expvs[ti % 2],
                                   hards[ti % 2], mpads[ti % 2])
            invni_t = small.tile([128, 1], F32, tag="invni")
            nc.gpsimd.dma_start(out=invni_t[:rt, :],
                                in_=invni_col[r0:r0 + rt, :])

            # S matmul + additive mask -> fp16 S16 (padded layout)
            for n in range(NCH):
                n0 = n * CHW
                sps = psS.tile([128, CHW], F32, tag="s")
                for kc in range(KC):
                    nc.tensor.matmul(sps[:mm, :],
                                     imgsT_sb[:, kc, r0:r0 + mm],
                                     caps_sb[:, kc, n0:n0 + CHW],
                                     start=(kc == 0), stop=False)
                nc.tensor.matmul(sps[:mm, :], ones_bf[:, :mm],
                                 addsrow_sb[:, n0:n0 + CHW],
                                 start=False, stop=True)
                nc.scalar.activation(
                    S16[:rt, n * 8 * WP:(n + 1) * 8 * WP].rearrange(
                        "p (c w) -> p c w", w=WP)[:, :, 0:W],
                    sps[:rt, :].rearrange("p (c w) -> p c w", w=W), AF.Copy)

            # rowmax tree + 2x duplication for the packed compare
            rmax = tree_reduce(S16, rt, OP.max, f"mx{ti % 2}")
            rdup = small.tile([128, 2 * BC], FP16, tag="rdup")
            nc.vector.tensor_copy(
                rdup[:rt, :].rearrange("p (c t) -> p c t", t=2),
                rmax[:rt, :, None].to_broadcast([rt, BC, 2]))

            # hard = (S16 == rowmax)   [2x_1p packed fp16 compare]
            nc.vector.tensor_tensor(
                vdup(hard, rt), vdup(S16, rt),
                rdup[:rt, :].rearrange("p (c t) -> p c t", t=2)[:, :, None, :]
                    .to_broadcast([rt, BC, W // 2, 2]),
                OP.is_equal)

            # exp on ScalarE into padded layout; den via tree
            nc.scalar.activation(vp(expv, rt), vp(S16, rt), AF.Exp,
                                 scale=invni_t[:rt, :])
            den = tree_reduce(expv, rt, OP.add, f"dn{ti % 2}")
            invden = small.tile([128, BC], F32, tag="invden")
            nc.vector.reciprocal(invden[:rt, :], den[:rt, :])
            idup = small.tile([128, 2 * BC], BF16, tag="idup")
            nc.vector.tensor_scalar_mul(
                idup[:rt, :].rearrange("p (c t) -> p c t", t=2),
                invden[:rt, :, None].to_broadcast([rt, BC, 2]), r_mix)

            # mixed = hard + (r/den)*exp, then *= nc*mask -> padded mixed
            nc.vector.tensor_tensor(
                vdup(expv, rt), vdup(expv, rt),
                idup[:rt, :].rearrange("p (c t) -> p c t", t=2)[:, :, None, :]
                    .to_broadcast([rt, BC, W // 2, 2]),
                OP.mult)
            nc.vector.tensor_tensor(vp(expv, rt), vp(expv, rt), vp(hard, rt),
                                    OP.add)
            nc.vector.tensor_tensor(vp(mp, rt), vp(expv, rt), vp(ncmask, rt),
                                    OP.mult)

            # num = sum_w mixed * S   (product into hard, tree reduce)
            nc.vector.tensor_tensor(vp(hard, rt), vp(mp, rt), vp(S16, rt),
                                    OP.mult)
            num = tree_reduce(hard, rt, OP.add, f"nm{ti % 2}")

            # all 32 pair-block transposes in one XBAR DMA
            M_T = work.tile([128, NP, 128], BF16, tag="MT")
            nc.sync.dma_start_transpose(out=M_T[:], in_=mp[:])
            state[ti] = dict(mp=mp, M_T=M_T, expv=expv, num=num)

        def tile_back(ti):
            r0, rt = ROW_TILES[ti]
            st = state.pop(ti)
            mp, M_T, expv = st["mp"], st["M_T"], st["expv"]
            # u = G * mixed per pair; 4 pairs per PSUM tile, evac to U
            U = work.tile([128, CWP], BF16, tag="U")
            for g in range(NP // 4):
                ups = psU.tile([128, 512], F32, tag="u")
                for jj in range(4):
                    j = 4 * g + jj
                    nc.tensor.matmul(ups[:, 128 * jj:128 * (jj + 1)],
                                     M_T[:, j, :], Gp[:, j, :],
                                     start=True, stop=True)
                nc.scalar.activation(U[:rt, 512 * g:512 * (g + 1)],
                                     ups[:rt, :], AF.Copy)

            # qf = sum_w mixed * u  (full width: mixed pads are zero)
            nc.vector.tensor_tensor(expv[:rt, :], mp[:rt, :], U[:rt, :],
                                    OP.mult)
            qf = tree_reduce(expv, rt, OP.add, f"qf{ti % 2}")

            nc.scalar.dma_start(out=out_num[r0:r0 + rt, :],
                                in_=st["num"][:rt, :])
            nc.scalar.dma_start(out=out_qf[r0:r0 + rt, :], in_=qf[:rt, :])

        for ti in range(len(ROW_TILES) + 1):
            if ti < len(ROW_TILES):
                tile_front(ti)
            if ti >= 1:
                tile_back(ti - 1)

    nc.finalize()
    return nc


def _get_runner(r_mix: float):
    key = round(float(r_mix), 9)
    if key not in _CACHE:
        _CACHE[key] = _build(key)
    return _CACHE[key]


def _host_prep(imgs, caps, img_lens, cap_lens):
    imgs = np.ascontiguousarray(np.asarray(imgs, dtype=np.float32))
    caps = np.ascontiguousarray(np.asarray(caps, dtype=np.float32))
    cap_lens = np.asarray(cap_lens).astype(np.int64)

    ncn = np.linalg.norm(caps, axis=-1) + EPS            # [Bc, W]
    cn = caps / ncn[..., None]
    cnb = cn.astype(ml_dtypes.bfloat16).astype(np.float32)
    # caps packed so each (kc, half) DMA piece is contiguous in DRAM
    capsT = cn.reshape(CW, D).T                          # [D, CW]
    caps_pk = np.ascontiguousarray(
        capsT.reshape(KC, 128, 2, CW // 2).transpose(0, 2, 1, 3)).astype(
        ml_dtypes.bfloat16)                              # [KC, 2, 128, 1600]
    cap_mask = (np.arange(W)[None, :] < cap_lens[:, None])  # [Bc, W]
    adds_row = np.where(cap_mask.reshape(1, CW), 0.0,
                        NEGS).astype(ml_dtypes.bfloat16)
    ncm_row = np.where(cap_mask, ncn, 0.0).reshape(1, CW).astype(
        ml_dtypes.bfloat16)
    inv_ni = (10.0 / (np.linalg.norm(imgs, axis=-1) + EPS)).astype(
        np.float32)                                      # [B, R]
    # pair-block-diagonal normalized caption Gram, computed on host:
    # gp_in[p, j, m]: G_{2j} at [0:50, j, 0:50], G_{2j+1} at [64:114, j, 64:]
    G = np.einsum('cwd,cvd->cwv', cnb, cnb)              # [Bc, W, W]
    gp = np.zeros((128, NP, 128), np.float32)
    gp[:W, :, :W] = G[0::2].transpose(1, 0, 2)
    gp[WP:WP + W, :, WP:WP + W] = G[1::2].transpose(1, 0, 2)
    gp_in = gp.astype(ml_dtypes.bfloat16)

    in_maps = []
    for core in range(N_CORES):
        sl = slice(core * BI, (core + 1) * BI)
        imT = imgs[sl].reshape(ROWS, D).T                # [D, ROWS]
        imgs_pk = np.ascontiguousarray(
            imT.reshape(KC, 128, ROWS).transpose(1, 0, 2)).astype(
            ml_dtypes.bfloat16)                          # [128, KC, ROWS]
        in_maps.append({
            "caps_pk": caps_pk,
            "gp_in": gp_in,
            "imgs_pk": imgs_pk,
            "adds_row": adds_row,
            "ncm_row": ncm_row,
            "invni_col": np.ascontiguousarray(
                inv_ni[sl].reshape(ROWS, 1)),
        })
    return in_maps


def run_on_device(inputs: dict, trace: bool = False):
    """Returns (output [64,64,36] f32, BassKernelResults)."""
    from concourse.bass_utils import run_bass_kernel_spmd
    alpha = float(np.asarray(inputs["alpha"]).reshape(-1)[0])
    a = 1.0 / (1.0 + np.exp(-alpha))
    r_mix = a / max(1.0 - a, 1e-9)
    nc = _get_runner(r_mix)
    in_maps = _host_prep(inputs["imgs"], inputs["caps"], inputs["img_lens"],
                         inputs["cap_lens"])
    r = run_bass_kernel_spmd(nc, in_maps, list(range(N_CORES)), trace=trace)
    img_lens = np.asarray(inputs["img_lens"]).astype(np.int64)
    iv = (np.arange(R)[None, :] < img_lens[:, None])     # [B, R]
    outs = []
    for c in range(N_CORES):
        num = r.results[c]["out_num"].astype(np.float32)  # [ROWS, BC]
        qf = r.results[c]["out_qf"].astype(np.float32)
        o = num / (np.sqrt(np.maximum(qf, 0.0)) + 1e-30)
        o = o.reshape(BI, R, BC).transpose(0, 2, 1)       # [BI, BC, R]
        o = np.where(iv[c * BI:(c + 1) * BI, None, :], o, -1.0)
        outs.append(o)
    return np.concatenate(outs, axis=0).astype(np.float32), r


def kernel(imgs, caps, img_lens, cap_lens, alpha):
    out, _ = run_on_device({"imgs": imgs, "caps": caps, "img_lens": img_lens,
                            "cap_lens": cap_lens, "alpha": alpha})
    return out


# revision 24
# speedup vs baseline: 1.3271x; 1.0530x over previous
"""Trainium2 Bass kernel for nn_AdaptiveMixedCoding (8 NeuronCores).

Sharding: data-parallel over B_img (8 images per core); caps/cap_lens/alpha
replicated; caption Gram precomputed on host and shipped (tiny BLAS work).

v3 design (per core: Bi=8 imgs, R=36 regions -> 288 rows; Bc=64 caps, W=50
words, D=1024):
  - caps pre-NORMALIZED on host -> S matmul yields cosine*|img|; additive
    word mask (-6e4) folded in as a K=1 ones-row matmul; 800-wide chunks so
    each PSUM evac covers exactly 16 captions and lands in a W=64-padded
    fp16 S16 layout [128, 64*64].
  - hard attention = is_equal(S16, rowmax) with rowmax duplicated 2x so the
    compare runs in DVE 2x_1p packed mode; fp16 keeps argmax ties rare.
  - exp on ScalarE (scale=10/|img| per row, into the padded layout); all
    per-caption reductions (rowmax/den/num/qf) run as binary trees over the
    padded layout, with the widest tree stage offloaded to the Pool engine.
  - mixed = hard + (r/den)*exp in bf16 2x ops, scaled by nc*mask into the
    padded mixed tile; one XBAR DMA transpose yields all 32 caption-pair
    blocks [128(cw), 32, 128(rows)]; 32 PE matmuls against the
    pair-block-diagonal Gram; qf = tree-reduce of mixed*u.
  - two-stage software pipeline: tile i's Gram/qf phase is emitted after
    tile i+1's softmax phase so no engine FIFO cross-blocks.
  - device outputs num and qf ([rows, 64] each); host does
    out = num/sqrt(qf), invalid-row masking, and the layout transpose.
"""
import sys
import contextlib

sys.path.insert(0, '/opt/trn_rl_repo')

import numpy as np
import ml_dtypes

from concourse import bacc, tile, mybir

F32 = mybir.dt.float32
BF16 = mybir.dt.bfloat16
FP16 = mybir.dt.float16
AF = mybir.ActivationFunctionType
OP = mybir.AluOpType
AX = mybir.AxisListType

N_CORES = 8
B, R, W, D = 64, 36, 50, 1024
BC = B
BI = B // N_CORES
ROWS = BI * R               # 288
CW = BC * W                 # 3200
WP = 64                     # padded word slot
CWP = BC * WP               # 4096
KC = D // 128               # 8 contraction chunks
NP = BC // 2                # 32 caption pairs
ROW_TILES = [(0, 108), (108, 108), (216, 72)]
NCH = 8                     # 400-wide S chunks = 8 captions each
CHW = 400
EPS = 1e-8
NEGS = -60000.0             # additive word mask; fp16-safe

_CACHE = {}


def _build(r_mix: float):
    nc = bacc.Bacc("TRN2", target_bir_lowering=False, debug=False,
                   num_devices=N_CORES)

    # caps packed as 16 contiguous pieces [kc, half] -> [128, 1600]
    caps_pk = nc.declare_dram_parameter("caps_pk", [KC, 2, 128, CW // 2],
                                        BF16, isOutput=False)
    gp_in = nc.declare_dram_parameter("gp_in", [128, NP, 128], BF16,
                                      isOutput=False)
    # imgs packed in the SBUF layout [128, KC, ROWS] (contiguous rows)
    imgs_pk = nc.declare_dram_parameter("imgs_pk", [128, KC, ROWS], BF16,
                                        isOutput=False)
    adds_row = nc.declare_dram_parameter("adds_row", [1, CW], BF16,
                                         isOutput=False)    # 0 / NEGS
    ncm_row = nc.declare_dram_parameter("ncm_row", [1, CW], BF16,
                                        isOutput=False)     # nc_w * mask01
    invni_col = nc.declare_dram_parameter("invni_col", [ROWS, 1], F32,
                                          isOutput=False)   # 10/|img row|
    out_num = nc.declare_dram_parameter("out_num", [ROWS, BC], F32,
                                        isOutput=True)
    out_qf = nc.declare_dram_parameter("out_qf", [ROWS, BC], F32,
                                       isOutput=True)

    with tile.TileContext(nc) as tc, contextlib.ExitStack() as ctx:
        const = ctx.enter_context(tc.tile_pool(name="const", bufs=1))
        big = ctx.enter_context(tc.tile_pool(name="big", bufs=1))
        work = ctx.enter_context(tc.tile_pool(name="work", bufs=2))
        small = ctx.enter_context(tc.tile_pool(name="small", bufs=2))
        tree = ctx.enter_context(tc.tile_pool(name="tree", bufs=3))
        psS = ctx.enter_context(tc.tile_pool(name="psS", bufs=3, space="PSUM"))
        psU = ctx.enter_context(tc.tile_pool(name="psU", bufs=2, space="PSUM"))
        psG = ctx.enter_context(tc.tile_pool(name="psG", bufs=1, space="PSUM"))

        ones_bf = const.tile([1, 128], BF16)
        nc.gpsimd.memset(ones_bf[:], 1.0)

        # ---- input loads ------------------------------------------------
        imgsT_sb = big.tile([128, KC, ROWS], BF16)
        nc.sync.dma_start(out=imgsT_sb[:], in_=imgs_pk[:])
        addsrow_sb = const.tile([1, CW], BF16)
        nc.gpsimd.dma_start(out=addsrow_sb[:], in_=adds_row[:])
        ncmrow_sb = const.tile([1, CW], BF16)
        nc.gpsimd.dma_start(out=ncmrow_sb[:], in_=ncm_row[:])
        Gp = big.tile([128, NP, 128], BF16)
        nc.gpsimd.dma_start(out=Gp[:], in_=gp_in[:])
        # caps: 16 contiguous pieces, alternating between the two HWDGE
        # queues; first column half fully arrives before the second
        caps_sb = big.tile([128, KC, CW], BF16)
        for h in range(2):
            for kc in range(KC):
                eng = nc.sync if (kc % 2 == 0) else nc.scalar
                eng.dma_start(
                    out=caps_sb[:, kc, h * 1600:(h + 1) * 1600],
                    in_=caps_pk[kc, h])

        # ---- broadcast nc*mask row into padded layout -------------------
        ncmask = big.tile([128, CWP], BF16)
        nc.vector.memset(
            ncmask.rearrange("p (c w) -> p c w", w=WP)[:, :, W:WP], 0.0)
        for n in range(NCH):
            bps = psG.tile([128, CHW], F32, tag="b")
            nc.tensor.matmul(bps[:], ones_bf[:],
                             ncmrow_sb[:, n * CHW:(n + 1) * CHW],
                             start=True, stop=True)
            nc.scalar.activation(
                ncmask[:, n * 8 * WP:(n + 1) * 8 * WP].rearrange(
                    "p (c w) -> p c w", w=WP)[:, :, 0:W],
                bps[:].rearrange("p (c w) -> p c w", w=W), AF.Copy)

        # persistent padded ping-pong tiles; pads written once, the loop
        # only ever touches the [64, 0:50] sub-views
        def pads(t):
            return t.rearrange("p (c w) -> p c w", w=WP)[:, :, W:WP]

        S16s, expvs, hards, mpads = [], [], [], []
        for i in range(2):
            s16 = big.tile([128, CWP], FP16, name=f"s16_{i}")
            nc.vector.memset(pads(s16), NEGS)
            S16s.append(s16)
            ev = big.tile([128, CWP], BF16, name=f"expv_{i}")
            nc.vector.memset(pads(ev), 0.0)
            expvs.append(ev)
            hd = big.tile([128, CWP], BF16, name=f"hard_{i}")
            nc.vector.memset(pads(hd), 0.0)
            hards.append(hd)
            # mixed pads feed the XBAR/u-matmuls: full memset
            mq = big.tile([128, CWP], BF16, name=f"mpad_{i}")
            nc.vector.memset(mq[:], 0.0)
            mpads.append(mq)

        def vp(t, rt):        # padded [rt, 64, 50] view
            return t[:rt, :].rearrange("p (c w) -> p c w", w=WP)[:, :, 0:W]

        def vdup(t, rt):      # padded [rt, 64, 25, 2] view (packed compare)
            return t[:rt, :].rearrange("p (c w) -> p c w", w=WP)[
                :, :, 0:W].rearrange("p c (k t) -> p c k t", t=2)

        def tree_reduce(x, rt, op, nm):
            # x: padded [128, CWP] tile; returns [128, BC] f32 tile.
            # binary tree stages; pads are identity elements.
            x3 = x[:rt, :].rearrange("p (c w) -> p c w", w=WP)
            a1 = tree.tile([128, BC * 32], x.dtype, tag="a1", name=f"a1{nm}")
            a13 = a1[:rt, :].rearrange("p (c w) -> p c w", w=32)
            nc.vector.tensor_tensor(a13, x3[:, :, 0:32], x3[:, :, 32:64], op)
            a2 = tree.tile([128, BC * 16], x.dtype, tag="a2", name=f"a2{nm}")
            a23 = a2[:rt, :].rearrange("p (c w) -> p c w", w=16)
            nc.vector.tensor_tensor(a23, a13[:, :, 0:16], a13[:, :, 16:32],
                                    op)
            out = small.tile([128, BC], F32, tag=f"tr{nm}", name=f"tr{nm}")
            nc.vector.tensor_reduce(out[:rt, :], a23, axis=AX.X, op=op)
            return out

        # ---- per row-tile software pipeline -----------------------------
        state = {}

        def tile_front(ti):
            r0, rt = ROW_TILES[ti]
            mm = 128 if r0 + 128 <= ROWS else rt
            S16, expv, hard, mp = (S16s[ti % 2], expvs[ti % 2],
                                   hards[ti % 2], mpads[ti % 2])
            invni_t = small.tile([128, 1], F32, tag="invni")
            nc.gpsimd.dma_start(out=invni_t[:rt, :],
                                in_=invni_col[r0:r0 + rt, :])

            # S matmul + additive mask -> fp16 S16 (padded layout)
            for n in range(NCH):
                n0 = n * CHW
                sps = psS.tile([128, CHW], F32, tag="s")
                for kc in range(KC):
                    nc.tensor.matmul(sps[:mm, :],
                                     imgsT_sb[:, kc, r0:r0 + mm],
                                     caps_sb[:, kc, n0:n0 + CHW],
                                     start=(kc == 0), stop=False)
                nc.tensor.matmul(sps[:mm, :], ones_bf[:, :mm],
                                 addsrow_sb[:, n0:n0 + CHW],
                                 start=False, stop=True)
                nc.scalar.activation(
                    S16[:rt, n * 8 * WP:(n + 1) * 8 * WP].rearrange(
                        "p (c w) -> p c w", w=WP)[:, :, 0:W],
                    sps[:rt, :].rearrange("p (c w) -> p c w", w=W), AF.Copy)

            # rowmax tree + 2x duplication for the packed compare
            rmax = tree_reduce(S16, rt, OP.max, f"mx{ti % 2}")
            rdup = small.tile([128, 2 * BC], FP16, tag="rdup")
            nc.vector.tensor_copy(
                rdup[:rt, :].rearrange("p (c t) -> p c t", t=2),
                rmax[:rt, :, None].to_broadcast([rt, BC, 2]))

            # hard = (S16 == rowmax)   [2x_1p packed fp16 compare]
            nc.vector.tensor_tensor(
                vdup(hard, rt), vdup(S16, rt),
                rdup[:rt, :].rearrange("p (c t) -> p c t", t=2)[:, :, None, :]
                    .to_broadcast([rt, BC, W // 2, 2]),
                OP.is_equal)

            # exp on ScalarE into padded layout; den via tree
            nc.scalar.activation(vp(expv, rt), vp(S16, rt), AF.Exp,
                                 scale=invni_t[:rt, :])
            den = tree_reduce(expv, rt, OP.add, f"dn{ti % 2}")
            invden = small.tile([128, BC], F32, tag="invden")
            nc.vector.reciprocal(invden[:rt, :], den[:rt, :])
            idup = small.tile([128, 2 * BC], BF16, tag="idup")
            nc.vector.tensor_scalar_mul(
                idup[:rt, :].rearrange("p (c t) -> p c t", t=2),
                invden[:rt, :, None].to_broadcast([rt, BC, 2]), r_mix)

            # mixed = hard + (r/den)*exp, then *= nc*mask -> padded mixed
            nc.vector.tensor_tensor(
                vdup(expv, rt), vdup(expv, rt),
                idup[:rt, :].rearrange("p (c t) -> p c t", t=2)[:, :, None, :]
                    .to_broadcast([rt, BC, W // 2, 2]),
                OP.mult)
            nc.vector.tensor_tensor(vp(expv, rt), vp(expv, rt), vp(hard, rt),
                                    OP.add)
            nc.vector.tensor_tensor(vp(mp, rt), vp(expv, rt), vp(ncmask, rt),
                                    OP.mult)

            # num = sum_w mixed * S   (product into hard, tree reduce)
            nc.vector.tensor_tensor(vp(hard, rt), vp(mp, rt), vp(S16, rt),
                                    OP.mult)
            num = tree_reduce(hard, rt, OP.add, f"nm{ti % 2}")

            # all 32 pair-block transposes in one XBAR DMA
            M_T = work.tile([128, NP, 128], BF16, tag="MT")
            nc.sync.dma_start_transpose(out=M_T[:], in_=mp[:])
            state[ti] = dict(mp=mp, M_T=M_T, expv=expv, num=num)

        def tile_back(ti):
            r0, rt = ROW_TILES[ti]
            st = state.pop(ti)
            mp, M_T, expv = st["mp"], st["M_T"], st["expv"]
            # u = G * mixed per pair; 4 pairs per PSUM tile, evac to U
            U = work.tile([128, CWP], BF16, tag="U")
            for g in range(NP // 4):
                ups = psU.tile([128, 512], F32, tag="u")
                for jj in range(4):
                    j = 4 * g + jj
                    nc.tensor.matmul(ups[:, 128 * jj:128 * (jj + 1)],
                                     M_T[:, j, :], Gp[:, j, :],
                                     start=True, stop=True)
                nc.scalar.activation(U[:rt, 512 * g:512 * (g + 1)],
                                     ups[:rt, :], AF.Copy)

            # qf = sum_w mixed * u  (50-col views run 2x; expv pads stay 0)
            nc.vector.tensor_tensor(vp(expv, rt), vp(mp, rt), vp(U, rt),
                                    OP.mult)
            qf = tree_reduce(expv, rt, OP.add, f"qf{ti % 2}")

            nc.scalar.dma_start(out=out_num[r0:r0 + rt, :],
                                in_=st["num"][:rt, :])
            nc.scalar.dma_start(out=out_qf[r0:r0 + rt, :], in_=qf[:rt, :])

        for ti in range(len(ROW_TILES) + 1):
            if ti < len(ROW_TILES):
                tile_front(ti)
            if ti >= 1:
                tile_back(ti - 1)

    nc.finalize()
    return nc


def _get_runner(r_mix: float):
    key = round(float(r_mix), 9)
    if key not in _CACHE:
        _CACHE[key] = _build(key)
    return _CACHE[key]


def _host_prep(imgs, caps, img_lens, cap_lens):
    imgs = np.ascontiguousarray(np.asarray(imgs, dtype=np.float32))
    caps = np.ascontiguousarray(np.asarray(caps, dtype=np.float32))
    cap_lens = np.asarray(cap_lens).astype(np.int64)

    ncn = np.linalg.norm(caps, axis=-1) + EPS            # [Bc, W]
    cn = caps / ncn[..., None]
    cnb = cn.astype(ml_dtypes.bfloat16).astype(np.float32)
    # caps packed so each (kc, half) DMA piece is contiguous in DRAM
    capsT = cn.reshape(CW, D).T                          # [D, CW]
    caps_pk = np.ascontiguousarray(
        capsT.reshape(KC, 128, 2, CW // 2).transpose(0, 2, 1, 3)).astype(
        ml_dtypes.bfloat16)                              # [KC, 2, 128, 1600]
    cap_mask = (np.arange(W)[None, :] < cap_lens[:, None])  # [Bc, W]
    adds_row = np.where(cap_mask.reshape(1, CW), 0.0,
                        NEGS).astype(ml_dtypes.bfloat16)
    ncm_row = np.where(cap_mask, ncn, 0.0).reshape(1, CW).astype(
        ml_dtypes.bfloat16)
    inv_ni = (10.0 / (np.linalg.norm(imgs, axis=-1) + EPS)).astype(
        np.float32)                                      # [B, R]
    # pair-block-diagonal normalized caption Gram, computed on host:
    # gp_in[p, j, m]: G_{2j} at [0:50, j, 0:50], G_{2j+1} at [64:114, j, 64:]
    G = np.einsum('cwd,cvd->cwv', cnb, cnb)              # [Bc, W, W]
    gp = np.zeros((128, NP, 128), np.float32)
    gp[:W, :, :W] = G[0::2].transpose(1, 0, 2)
    gp[WP:WP + W, :, WP:WP + W] = G[1::2].transpose(1, 0, 2)
    gp_in = gp.astype(ml_dtypes.bfloat16)

    in_maps = []
    for core in range(N_CORES):
        sl = slice(core * BI, (core + 1) * BI)
        imT = imgs[sl].reshape(ROWS, D).T                # [D, ROWS]
        imgs_pk = np.ascontiguousarray(
            imT.reshape(KC, 128, ROWS).transpose(1, 0, 2)).astype(
            ml_dtypes.bfloat16)                          # [128, KC, ROWS]
        in_maps.append({
            "caps_pk": caps_pk,
            "gp_in": gp_in,
            "imgs_pk": imgs_pk,
            "adds_row": adds_row,
            "ncm_row": ncm_row,
            "invni_col": np.ascontiguousarray(
                inv_ni[sl].reshape(ROWS, 1)),
        })
    return in_maps


def run_on_device(inputs: dict, trace: bool = False):
    """Returns (output [64,64,36] f32, BassKernelResults)."""
    from concourse.bass_utils import run_bass_kernel_spmd
    alpha = float(np.asarray(inputs["alpha"]).reshape(-1)[0])
    a = 1.0 / (1.0 + np.exp(-alpha))
    r_mix = a / max(1.0 - a, 1e-9)
    nc = _get_runner(r_mix)
    in_maps = _host_prep(inputs["imgs"], inputs["caps"], inputs["img_lens"],
                         inputs["cap_lens"])
    r = run_bass_kernel_spmd(nc, in_maps, list(range(N_CORES)), trace=trace)
    img_lens = np.asarray(inputs["img_lens"]).astype(np.int64)
    iv = (np.arange(R)[None, :] < img_lens[:, None])     # [B, R]
    outs = []
    for c in range(N_CORES):
        num = r.results[c]["out_num"].astype(np.float32)  # [ROWS, BC]
        qf = r.results[c]["out_qf"].astype(np.float32)
        o = num / (np.sqrt(np.maximum(qf, 0.0)) + 1e-30)
        o = o.reshape(BI, R, BC).transpose(0, 2, 1)       # [BI, BC, R]
        o = np.where(iv[c * BI:(c + 1) * BI, None, :], o, -1.0)
        outs.append(o)
    return np.concatenate(outs, axis=0).astype(np.float32), r


def kernel(imgs, caps, img_lens, cap_lens, alpha):
    out, _ = run_on_device({"imgs": imgs, "caps": caps, "img_lens": img_lens,
                            "cap_lens": cap_lens, "alpha": alpha})
    return out


# revision 25
# speedup vs baseline: 1.3826x; 1.0418x over previous
"""Trainium2 Bass kernel for nn_AdaptiveMixedCoding (8 NeuronCores).

Sharding: data-parallel over B_img (8 images per core); caps/cap_lens/alpha
replicated; caption Gram precomputed on host and shipped (tiny BLAS work).

v3 design (per core: Bi=8 imgs, R=36 regions -> 288 rows; Bc=64 caps, W=50
words, D=1024):
  - caps pre-NORMALIZED on host -> S matmul yields cosine*|img|; additive
    word mask (-6e4) folded in as a K=1 ones-row matmul; 800-wide chunks so
    each PSUM evac covers exactly 16 captions and lands in a W=64-padded
    fp16 S16 layout [128, 64*64].
  - hard attention = is_equal(S16, rowmax) with rowmax duplicated 2x so the
    compare runs in DVE 2x_1p packed mode; fp16 keeps argmax ties rare.
  - exp on ScalarE (scale=10/|img| per row, into the padded layout); all
    per-caption reductions (rowmax/den/num/qf) run as binary trees over the
    padded layout, with the widest tree stage offloaded to the Pool engine.
  - mixed = hard + (r/den)*exp in bf16 2x ops, scaled by nc*mask into the
    padded mixed tile; one XBAR DMA transpose yields all 32 caption-pair
    blocks [128(cw), 32, 128(rows)]; 32 PE matmuls against the
    pair-block-diagonal Gram; qf = tree-reduce of mixed*u.
  - two-stage software pipeline: tile i's Gram/qf phase is emitted after
    tile i+1's softmax phase so no engine FIFO cross-blocks.
  - device outputs num and qf ([rows, 64] each); host does
    out = num/sqrt(qf), invalid-row masking, and the layout transpose.
"""
import sys
import contextlib

sys.path.insert(0, '/opt/trn_rl_repo')

import numpy as np
import ml_dtypes

from concourse import bacc, tile, mybir

F32 = mybir.dt.float32
BF16 = mybir.dt.bfloat16
FP16 = mybir.dt.float16
AF = mybir.ActivationFunctionType
OP = mybir.AluOpType
AX = mybir.AxisListType

N_CORES = 8
B, R, W, D = 64, 36, 50, 1024
BC = B
BI = B // N_CORES
ROWS = BI * R               # 288
CW = BC * W                 # 3200
WP = 64                     # padded word slot
CWP = BC * WP               # 4096
KC = D // 128               # 8 contraction chunks
NP = BC // 2                # 32 caption pairs
ROW_TILES = [(0, 108), (108, 108), (216, 72)]
NCH = 8                     # 400-wide S chunks = 8 captions each
CHW = 400
EPS = 1e-8
NEGS = -60000.0             # additive word mask; fp16-safe

_CACHE = {}


def _build(r_mix: float):
    nc = bacc.Bacc("TRN2", target_bir_lowering=False, debug=False,
                   num_devices=N_CORES)

    # caps packed as 16 contiguous pieces [kc, half] -> [128, 1600]
    caps_pk = nc.declare_dram_parameter("caps_pk", [KC, 2, 128, CW // 2],
                                        BF16, isOutput=False)
    gp_in = nc.declare_dram_parameter("gp_in", [128, NP, 128], BF16,
                                      isOutput=False)
    # imgs packed in the SBUF layout [128, KC, ROWS] (contiguous rows)
    imgs_pk = nc.declare_dram_parameter("imgs_pk", [128, KC, ROWS], BF16,
                                        isOutput=False)
    adds_row = nc.declare_dram_parameter("adds_row", [1, CW], BF16,
                                         isOutput=False)    # 0 / NEGS
    ncm_row = nc.declare_dram_parameter("ncm_row", [1, CW], BF16,
                                        isOutput=False)     # nc_w * mask01
    invni_col = nc.declare_dram_parameter("invni_col", [ROWS, 1], F32,
                                          isOutput=False)   # 10/|img row|
    out_num = nc.declare_dram_parameter("out_num", [ROWS, BC], F32,
                                        isOutput=True)
    out_qf = nc.declare_dram_parameter("out_qf", [ROWS, BC], F32,
                                       isOutput=True)

    with tile.TileContext(nc) as tc, contextlib.ExitStack() as ctx:
        const = ctx.enter_context(tc.tile_pool(name="const", bufs=1))
        big = ctx.enter_context(tc.tile_pool(name="big", bufs=1))
        work = ctx.enter_context(tc.tile_pool(name="work", bufs=2))
        small = ctx.enter_context(tc.tile_pool(name="small", bufs=2))
        tree = ctx.enter_context(tc.tile_pool(name="tree", bufs=3))
        psS = ctx.enter_context(tc.tile_pool(name="psS", bufs=3, space="PSUM"))
        psU = ctx.enter_context(tc.tile_pool(name="psU", bufs=3, space="PSUM"))
        psG = ctx.enter_context(tc.tile_pool(name="psG", bufs=1, space="PSUM"))

        ones_bf = const.tile([1, 128], BF16)
        nc.gpsimd.memset(ones_bf[:], 1.0)

        # ---- input loads ------------------------------------------------
        imgsT_sb = big.tile([128, KC, ROWS], BF16)
        nc.sync.dma_start(out=imgsT_sb[:], in_=imgs_pk[:])
        addsrow_sb = const.tile([1, CW], BF16)
        nc.gpsimd.dma_start(out=addsrow_sb[:], in_=adds_row[:])
        ncmrow_sb = const.tile([1, CW], BF16)
        nc.gpsimd.dma_start(out=ncmrow_sb[:], in_=ncm_row[:])
        Gp = big.tile([128, NP, 128], BF16)
        nc.gpsimd.dma_start(out=Gp[:], in_=gp_in[:])
        # caps: 16 contiguous pieces, alternating between the two HWDGE
        # queues; first column half fully arrives before the second
        caps_sb = big.tile([128, KC, CW], BF16)
        for h in range(2):
            for kc in range(KC):
                eng = nc.sync if (kc % 2 == 0) else nc.scalar
                eng.dma_start(
                    out=caps_sb[:, kc, h * 1600:(h + 1) * 1600],
                    in_=caps_pk[kc, h])

        # ---- broadcast nc*mask row into padded layout -------------------
        ncmask = big.tile([128, CWP], BF16)
        nc.vector.memset(
            ncmask.rearrange("p (c w) -> p c w", w=WP)[:, :, W:WP], 0.0)
        for n in range(NCH):
            bps = psG.tile([128, CHW], F32, tag="b")
            nc.tensor.matmul(bps[:], ones_bf[:],
                             ncmrow_sb[:, n * CHW:(n + 1) * CHW],
                             start=True, stop=True)
            nc.scalar.activation(
                ncmask[:, n * 8 * WP:(n + 1) * 8 * WP].rearrange(
                    "p (c w) -> p c w", w=WP)[:, :, 0:W],
                bps[:].rearrange("p (c w) -> p c w", w=W), AF.Copy)

        # persistent padded ping-pong tiles; pads written once, the loop
        # only ever touches the [64, 0:50] sub-views
        def pads(t):
            return t.rearrange("p (c w) -> p c w", w=WP)[:, :, W:WP]

        S16s, expvs, hards, mpads = [], [], [], []
        for i in range(2):
            s16 = big.tile([128, CWP], FP16, name=f"s16_{i}")
            nc.vector.memset(pads(s16), NEGS)
            S16s.append(s16)
            ev = big.tile([128, CWP], BF16, name=f"expv_{i}")
            nc.vector.memset(pads(ev), 0.0)
            expvs.append(ev)
            hd = big.tile([128, CWP], BF16, name=f"hard_{i}")
            nc.vector.memset(pads(hd), 0.0)
            hards.append(hd)
            # mixed pads feed the XBAR/u-matmuls: full memset
            mq = big.tile([128, CWP], BF16, name=f"mpad_{i}")
            nc.vector.memset(mq[:], 0.0)
            mpads.append(mq)

        def vp(t, rt):        # padded [rt, 64, 50] view
            return t[:rt, :].rearrange("p (c w) -> p c w", w=WP)[:, :, 0:W]

        def vdup(t, rt):      # padded [rt, 64, 25, 2] view (packed compare)
            return t[:rt, :].rearrange("p (c w) -> p c w", w=WP)[
                :, :, 0:W].rearrange("p c (k t) -> p c k t", t=2)

        def tree_reduce(x, rt, op, nm):
            # x: padded [128, CWP] tile; returns [128, BC] f32 tile.
            # binary tree stages; pads are identity elements.
            x3 = x[:rt, :].rearrange("p (c w) -> p c w", w=WP)
            a1 = tree.tile([128, BC * 32], x.dtype, tag="a1", name=f"a1{nm}")
            a13 = a1[:rt, :].rearrange("p (c w) -> p c w", w=32)
            nc.vector.tensor_tensor(a13, x3[:, :, 0:32], x3[:, :, 32:64], op)
            a2 = tree.tile([128, BC * 16], x.dtype, tag="a2", name=f"a2{nm}")
            a23 = a2[:rt, :].rearrange("p (c w) -> p c w", w=16)
            nc.vector.tensor_tensor(a23, a13[:, :, 0:16], a13[:, :, 16:32],
                                    op)
            out = small.tile([128, BC], F32, tag=f"tr{nm}", name=f"tr{nm}")
            nc.vector.tensor_reduce(out[:rt, :], a23, axis=AX.X, op=op)
            return out

        # ---- per row-tile software pipeline -----------------------------
        state = {}

        def tile_front(ti):
            r0, rt = ROW_TILES[ti]
            mm = 128 if r0 + 128 <= ROWS else rt
            S16, expv, hard, mp = (S16s[ti % 2], expvs[ti % 2],
                                   hards[ti % 2], mpads[ti % 2])
            invni_t = small.tile([128, 1], F32, tag="invni")
            nc.gpsimd.dma_start(out=invni_t[:rt, :],
                                in_=invni_col[r0:r0 + rt, :])

            # S matmul + additive mask -> fp16 S16 (padded layout)
            for n in range(NCH):
                n0 = n * CHW
                sps = psS.tile([128, CHW], F32, tag="s")
                for kc in range(KC):
                    nc.tensor.matmul(sps[:mm, :],
                                     imgsT_sb[:, kc, r0:r0 + mm],
                                     caps_sb[:, kc, n0:n0 + CHW],
                                     start=(kc == 0), stop=False)
                nc.tensor.matmul(sps[:mm, :], ones_bf[:, :mm],
                                 addsrow_sb[:, n0:n0 + CHW],
                                 start=False, stop=True)
                nc.scalar.activation(
                    S16[:rt, n * 8 * WP:(n + 1) * 8 * WP].rearrange(
                        "p (c w) -> p c w", w=WP)[:, :, 0:W],
                    sps[:rt, :].rearrange("p (c w) -> p c w", w=W), AF.Copy)

            # rowmax tree + 2x duplication for the packed compare
            rmax = tree_reduce(S16, rt, OP.max, f"mx{ti % 2}")
            rdup = small.tile([128, 2 * BC], FP16, tag="rdup")
            nc.vector.tensor_copy(
                rdup[:rt, :].rearrange("p (c t) -> p c t", t=2),
                rmax[:rt, :, None].to_broadcast([rt, BC, 2]))

            # hard = (S16 == rowmax)   [2x_1p packed fp16 compare]
            nc.vector.tensor_tensor(
                vdup(hard, rt), vdup(S16, rt),
                rdup[:rt, :].rearrange("p (c t) -> p c t", t=2)[:, :, None, :]
                    .to_broadcast([rt, BC, W // 2, 2]),
                OP.is_equal)

            # exp on ScalarE into padded layout; den via tree
            nc.scalar.activation(vp(expv, rt), vp(S16, rt), AF.Exp,
                                 scale=invni_t[:rt, :])
            den = tree_reduce(expv, rt, OP.add, f"dn{ti % 2}")
            invden = small.tile([128, BC], F32, tag="invden")
            nc.vector.reciprocal(invden[:rt, :], den[:rt, :])
            idup = small.tile([128, 2 * BC], BF16, tag="idup")
            nc.vector.tensor_scalar_mul(
                idup[:rt, :].rearrange("p (c t) -> p c t", t=2),
                invden[:rt, :, None].to_broadcast([rt, BC, 2]), r_mix)

            # mixed = hard + (r/den)*exp, then *= nc*mask -> padded mixed
            nc.vector.tensor_tensor(
                vdup(expv, rt), vdup(expv, rt),
                idup[:rt, :].rearrange("p (c t) -> p c t", t=2)[:, :, None, :]
                    .to_broadcast([rt, BC, W // 2, 2]),
                OP.mult)
            nc.vector.tensor_tensor(vp(expv, rt), vp(expv, rt), vp(hard, rt),
                                    OP.add)
            nc.vector.tensor_tensor(vp(mp, rt), vp(expv, rt), vp(ncmask, rt),
                                    OP.mult)

            # num = sum_w mixed * S   (product into hard, tree reduce)
            nc.vector.tensor_tensor(vp(hard, rt), vp(mp, rt), vp(S16, rt),
                                    OP.mult)
            num = tree_reduce(hard, rt, OP.add, f"nm{ti % 2}")

            # all 32 pair-block transposes in one XBAR DMA
            M_T = work.tile([128, NP, 128], BF16, tag="MT")
            nc.sync.dma_start_transpose(out=M_T[:], in_=mp[:])
            state[ti] = dict(mp=mp, M_T=M_T, expv=expv, num=num)

        def tile_back(ti):
            r0, rt = ROW_TILES[ti]
            st = state.pop(ti)
            mp, M_T, expv = st["mp"], st["M_T"], st["expv"]
            # u = G * mixed per pair; 4 pairs per PSUM tile, evac to U
            U = work.tile([128, CWP], BF16, tag="U")
            def vg(t, g):     # group g's 8 caption-slots, 50-col views
                return t[:rt, 512 * g:512 * (g + 1)].rearrange(
                    "p (c w) -> p c w", w=WP)[:, :, 0:W]

            for g in range(NP // 4):
                ups = psU.tile([128, 512], F32, tag="u")
                for jj in range(4):
                    j = 4 * g + jj
                    nc.tensor.matmul(ups[:, 128 * jj:128 * (jj + 1)],
                                     M_T[:, j, :], Gp[:, j, :],
                                     start=True, stop=True)
                nc.scalar.activation(U[:rt, 512 * g:512 * (g + 1)],
                                     ups[:rt, :], AF.Copy)
                # qf product for this group right after its evac
                nc.vector.tensor_tensor(vg(expv, g), vg(mp, g), vg(U, g),
                                        OP.mult)
            qf = tree_reduce(expv, rt, OP.add, f"qf{ti % 2}")

            nc.scalar.dma_start(out=out_num[r0:r0 + rt, :],
                                in_=st["num"][:rt, :])
            nc.scalar.dma_start(out=out_qf[r0:r0 + rt, :], in_=qf[:rt, :])

        for ti in range(len(ROW_TILES) + 1):
            if ti < len(ROW_TILES):
                tile_front(ti)
            if ti >= 1:
                tile_back(ti - 1)

    nc.finalize()
    return nc


def _get_runner(r_mix: float):
    key = round(float(r_mix), 9)
    if key not in _CACHE:
        _CACHE[key] = _build(key)
    return _CACHE[key]


def _host_prep(imgs, caps, img_lens, cap_lens):
    imgs = np.ascontiguousarray(np.asarray(imgs, dtype=np.float32))
    caps = np.ascontiguousarray(np.asarray(caps, dtype=np.float32))
    cap_lens = np.asarray(cap_lens).astype(np.int64)

    ncn = np.linalg.norm(caps, axis=-1) + EPS            # [Bc, W]
    cn = caps / ncn[..., None]
    cnb = cn.astype(ml_dtypes.bfloat16).astype(np.float32)
    # caps packed so each (kc, half) DMA piece is contiguous in DRAM
    capsT = cn.reshape(CW, D).T                          # [D, CW]
    caps_pk = np.ascontiguousarray(
        capsT.reshape(KC, 128, 2, CW // 2).transpose(0, 2, 1, 3)).astype(
        ml_dtypes.bfloat16)                              # [KC, 2, 128, 1600]
    cap_mask = (np.arange(W)[None, :] < cap_lens[:, None])  # [Bc, W]
    adds_row = np.where(cap_mask.reshape(1, CW), 0.0,
                        NEGS).astype(ml_dtypes.bfloat16)
    ncm_row = np.where(cap_mask, ncn, 0.0).reshape(1, CW).astype(
        ml_dtypes.bfloat16)
    inv_ni = (10.0 / (np.linalg.norm(imgs, axis=-1) + EPS)).astype(
        np.float32)                                      # [B, R]
    # pair-block-diagonal normalized caption Gram, computed on host:
    # gp_in[p, j, m]: G_{2j} at [0:50, j, 0:50], G_{2j+1} at [64:114, j, 64:]
    G = np.einsum('cwd,cvd->cwv', cnb, cnb)              # [Bc, W, W]
    gp = np.zeros((128, NP, 128), np.float32)
    gp[:W, :, :W] = G[0::2].transpose(1, 0, 2)
    gp[WP:WP + W, :, WP:WP + W] = G[1::2].transpose(1, 0, 2)
    gp_in = gp.astype(ml_dtypes.bfloat16)

    in_maps = []
    for core in range(N_CORES):
        sl = slice(core * BI, (core + 1) * BI)
        imT = imgs[sl].reshape(ROWS, D).T                # [D, ROWS]
        imgs_pk = np.ascontiguousarray(
            imT.reshape(KC, 128, ROWS).transpose(1, 0, 2)).astype(
            ml_dtypes.bfloat16)                          # [128, KC, ROWS]
        in_maps.append({
            "caps_pk": caps_pk,
            "gp_in": gp_in,
            "imgs_pk": imgs_pk,
            "adds_row": adds_row,
            "ncm_row": ncm_row,
            "invni_col": np.ascontiguousarray(
                inv_ni[sl].reshape(ROWS, 1)),
        })
    return in_maps


def run_on_device(inputs: dict, trace: bool = False):
    """Returns (output [64,64,36] f32, BassKernelResults)."""
    from concourse.bass_utils import run_bass_kernel_spmd
    alpha = float(np.asarray(inputs["alpha"]).reshape(-1)[0])
    a = 1.0 / (1.0 + np.exp(-alpha))
    r_mix = a / max(1.0 - a, 1e-9)
    nc = _get_runner(r_mix)
    in_maps = _host_prep(inputs["imgs"], inputs["caps"], inputs["img_lens"],
                         inputs["cap_lens"])
    r = run_bass_kernel_spmd(nc, in_maps, list(range(N_CORES)), trace=trace)
    img_lens = np.asarray(inputs["img_lens"]).astype(np.int64)
    iv = (np.arange(R)[None, :] < img_lens[:, None])     # [B, R]
    outs = []
    for c in range(N_CORES):
        num = r.results[c]["out_num"].astype(np.float32)  # [ROWS, BC]
        qf = r.results[c]["out_qf"].astype(np.float32)
        o = num / (np.sqrt(np.maximum(qf, 0.0)) + 1e-30)
        o = o.reshape(BI, R, BC).transpose(0, 2, 1)       # [BI, BC, R]
        o = np.where(iv[c * BI:(c + 1) * BI, None, :], o, -1.0)
        outs.append(o)
    return np.concatenate(outs, axis=0).astype(np.float32), r


def kernel(imgs, caps, img_lens, cap_lens, alpha):
    out, _ = run_on_device({"imgs": imgs, "caps": caps, "img_lens": img_lens,
                            "cap_lens": cap_lens, "alpha": alpha})
    return out


# revision 26
# speedup vs baseline: 1.3983x; 1.0113x over previous
"""Trainium2 Bass kernel for nn_AdaptiveMixedCoding (8 NeuronCores).

Sharding: data-parallel over B_img (8 images per core); caps/cap_lens/alpha
replicated; caption Gram precomputed on host and shipped (tiny BLAS work).

v3 design (per core: Bi=8 imgs, R=36 regions -> 288 rows; Bc=64 caps, W=50
words, D=1024):
  - caps pre-NORMALIZED on host -> S matmul yields cosine*|img|; additive
    word mask (-6e4) folded in as a K=1 ones-row matmul; 800-wide chunks so
    each PSUM evac covers exactly 16 captions and lands in a W=64-padded
    fp16 S16 layout [128, 64*64].
  - hard attention = is_equal(S16, rowmax) with rowmax duplicated 2x so the
    compare runs in DVE 2x_1p packed mode; fp16 keeps argmax ties rare.
  - exp on ScalarE (scale=10/|img| per row, into the padded layout); all
    per-caption reductions (rowmax/den/num/qf) run as binary trees over the
    padded layout, with the widest tree stage offloaded to the Pool engine.
  - mixed = hard + (r/den)*exp in bf16 2x ops, scaled by nc*mask into the
    padded mixed tile; one XBAR DMA transpose yields all 32 caption-pair
    blocks [128(cw), 32, 128(rows)]; 32 PE matmuls against the
    pair-block-diagonal Gram; qf = tree-reduce of mixed*u.
  - two-stage software pipeline: tile i's Gram/qf phase is emitted after
    tile i+1's softmax phase so no engine FIFO cross-blocks.
  - device outputs num and qf ([rows, 64] each); host does
    out = num/sqrt(qf), invalid-row masking, and the layout transpose.
"""
import sys
import contextlib

sys.path.insert(0, '/opt/trn_rl_repo')

import numpy as np
import ml_dtypes

from concourse import bacc, tile, mybir

F32 = mybir.dt.float32
BF16 = mybir.dt.bfloat16
FP16 = mybir.dt.float16
AF = mybir.ActivationFunctionType
OP = mybir.AluOpType
AX = mybir.AxisListType

N_CORES = 8
B, R, W, D = 64, 36, 50, 1024
BC = B
BI = B // N_CORES
ROWS = BI * R               # 288
CW = BC * W                 # 3200
WP = 64                     # padded word slot
CWP = BC * WP               # 4096
KC = D // 128               # 8 contraction chunks
NP = BC // 2                # 32 caption pairs
ROW_TILES = [(0, 108), (108, 108), (216, 72)]
NCH = 8                     # 400-wide S chunks = 8 captions each
CHW = 400
EPS = 1e-8
NEGS = -60000.0             # additive word mask; fp16-safe

_CACHE = {}


def _build(r_mix: float):
    nc = bacc.Bacc("TRN2", target_bir_lowering=False, debug=False,
                   num_devices=N_CORES)

    # caps packed as 16 contiguous pieces [kc, half] -> [128, 1600]
    caps_pk = nc.declare_dram_parameter("caps_pk", [KC, 2, 128, CW // 2],
                                        BF16, isOutput=False)
    gp_in = nc.declare_dram_parameter("gp_in", [128, NP, 128], BF16,
                                      isOutput=False)
    # imgs packed in the SBUF layout [128, KC, ROWS] (contiguous rows)
    imgs_pk = nc.declare_dram_parameter("imgs_pk", [128, KC, ROWS], BF16,
                                        isOutput=False)
    adds_row = nc.declare_dram_parameter("adds_row", [1, CW], BF16,
                                         isOutput=False)    # 0 / NEGS
    ncm_row = nc.declare_dram_parameter("ncm_row", [1, CW], BF16,
                                        isOutput=False)     # nc_w * mask01
    invni_col = nc.declare_dram_parameter("invni_col", [ROWS, 1], F32,
                                          isOutput=False)   # 10/|img row|
    out_num = nc.declare_dram_parameter("out_num", [ROWS, BC], F32,
                                        isOutput=True)
    out_qf = nc.declare_dram_parameter("out_qf", [ROWS, BC], F32,
                                       isOutput=True)

    with tile.TileContext(nc) as tc, contextlib.ExitStack() as ctx:
        const = ctx.enter_context(tc.tile_pool(name="const", bufs=1))
        big = ctx.enter_context(tc.tile_pool(name="big", bufs=1))
        work = ctx.enter_context(tc.tile_pool(name="work", bufs=2))
        small = ctx.enter_context(tc.tile_pool(name="small", bufs=2))
        tree = ctx.enter_context(tc.tile_pool(name="tree", bufs=3))
        psS = ctx.enter_context(tc.tile_pool(name="psS", bufs=3, space="PSUM"))
        psU = ctx.enter_context(tc.tile_pool(name="psU", bufs=3, space="PSUM"))
        psG = ctx.enter_context(tc.tile_pool(name="psG", bufs=1, space="PSUM"))

        ones_bf = const.tile([1, 128], BF16)
        nc.gpsimd.memset(ones_bf[:], 1.0)

        # ---- input loads ------------------------------------------------
        imgsT_sb = big.tile([128, KC, ROWS], BF16)
        nc.sync.dma_start(out=imgsT_sb[:], in_=imgs_pk[:])
        addsrow_sb = const.tile([1, CW], BF16)
        nc.gpsimd.dma_start(out=addsrow_sb[:], in_=adds_row[:])
        ncmrow_sb = const.tile([1, CW], BF16)
        nc.gpsimd.dma_start(out=ncmrow_sb[:], in_=ncm_row[:])
        Gp = big.tile([128, NP, 128], BF16)
        nc.gpsimd.dma_start(out=Gp[:], in_=gp_in[:])
        # caps: 16 contiguous pieces, alternating between the two HWDGE
        # queues; first column half fully arrives before the second
        caps_sb = big.tile([128, KC, CW], BF16)
        for h in range(2):
            for kc in range(KC):
                eng = nc.sync if (kc % 2 == 0) else nc.scalar
                eng.dma_start(
                    out=caps_sb[:, kc, h * 1600:(h + 1) * 1600],
                    in_=caps_pk[kc, h])

        # ---- broadcast nc*mask row into padded layout -------------------
        ncmask = big.tile([128, CWP], BF16)
        nc.vector.memset(
            ncmask.rearrange("p (c w) -> p c w", w=WP)[:, :, W:WP], 0.0)
        for n in range(NCH):
            bps = psG.tile([128, CHW], F32, tag="b")
            nc.tensor.matmul(bps[:], ones_bf[:],
                             ncmrow_sb[:, n * CHW:(n + 1) * CHW],
                             start=True, stop=True)
            nc.scalar.activation(
                ncmask[:, n * 8 * WP:(n + 1) * 8 * WP].rearrange(
                    "p (c w) -> p c w", w=WP)[:, :, 0:W],
                bps[:].rearrange("p (c w) -> p c w", w=W), AF.Copy)

        # persistent padded ping-pong tiles; pads written once, the loop
        # only ever touches the [64, 0:50] sub-views
        def pads(t):
            return t.rearrange("p (c w) -> p c w", w=WP)[:, :, W:WP]

        S16s, expvs, hards, mpads = [], [], [], []
        for i in range(2):
            s16 = big.tile([128, CWP], FP16, name=f"s16_{i}")
            nc.vector.memset(pads(s16), NEGS)
            S16s.append(s16)
            ev = big.tile([128, CWP], BF16, name=f"expv_{i}")
            nc.vector.memset(pads(ev), 0.0)
            expvs.append(ev)
            hd = big.tile([128, CWP], BF16, name=f"hard_{i}")
            nc.vector.memset(pads(hd), 0.0)
            hards.append(hd)
            # mixed pads feed the XBAR/u-matmuls: full memset
            mq = big.tile([128, CWP], BF16, name=f"mpad_{i}")
            nc.vector.memset(mq[:], 0.0)
            mpads.append(mq)

        def vp(t, rt):        # padded [rt, 64, 50] view
            return t[:rt, :].rearrange("p (c w) -> p c w", w=WP)[:, :, 0:W]

        def vdup(t, rt):      # padded [rt, 64, 25, 2] view (packed compare)
            return t[:rt, :].rearrange("p (c w) -> p c w", w=WP)[
                :, :, 0:W].rearrange("p c (k t) -> p c k t", t=2)

        def tree_reduce(x, rt, op, nm):
            # x: padded [128, CWP] tile; returns [128, BC] f32 tile.
            # binary tree stages; pads are identity elements.
            x3 = x[:rt, :].rearrange("p (c w) -> p c w", w=WP)
            a1 = tree.tile([128, BC * 32], x.dtype, tag="a1", name=f"a1{nm}")
            a13 = a1[:rt, :].rearrange("p (c w) -> p c w", w=32)
            nc.vector.tensor_tensor(a13, x3[:, :, 0:32], x3[:, :, 32:64], op)
            a2 = tree.tile([128, BC * 16], x.dtype, tag="a2", name=f"a2{nm}")
            a23 = a2[:rt, :].rearrange("p (c w) -> p c w", w=16)
            nc.vector.tensor_tensor(a23, a13[:, :, 0:16], a13[:, :, 16:32],
                                    op)
            out = small.tile([128, BC], F32, tag=f"tr{nm}", name=f"tr{nm}")
            nc.vector.tensor_reduce(out[:rt, :], a23, axis=AX.X, op=op)
            return out

        # ---- per row-tile software pipeline -----------------------------
        state = {}

        def tile_front(ti):
            r0, rt = ROW_TILES[ti]
            mm = 128 if r0 + 128 <= ROWS else rt
            S16, expv, hard, mp = (S16s[ti % 2], expvs[ti % 2],
                                   hards[ti % 2], mpads[ti % 2])
            invni_t = small.tile([128, 1], F32, tag="invni")
            nc.gpsimd.dma_start(out=invni_t[:rt, :],
                                in_=invni_col[r0:r0 + rt, :])

            # S matmul + additive mask -> fp16 S16 (padded layout)
            for n in range(NCH):
                n0 = n * CHW
                sps = psS.tile([128, CHW], F32, tag="s")
                for kc in range(KC):
                    nc.tensor.matmul(sps[:mm, :],
                                     imgsT_sb[:, kc, r0:r0 + mm],
                                     caps_sb[:, kc, n0:n0 + CHW],
                                     start=(kc == 0), stop=False)
                nc.tensor.matmul(sps[:mm, :], ones_bf[:, :mm],
                                 addsrow_sb[:, n0:n0 + CHW],
                                 start=False, stop=True)
                cs = slice(n * 8 * WP, (n + 1) * 8 * WP)
                nc.scalar.activation(
                    S16[:rt, cs].rearrange(
                        "p (c w) -> p c w", w=WP)[:, :, 0:W],
                    sps[:rt, :].rearrange("p (c w) -> p c w", w=W), AF.Copy)
                # exp for this chunk right away (overlaps later chunks)
                nc.scalar.activation(
                    expv[:rt, cs].rearrange(
                        "p (c w) -> p c w", w=WP)[:, :, 0:W],
                    S16[:rt, cs].rearrange(
                        "p (c w) -> p c w", w=WP)[:, :, 0:W],
                    AF.Exp, scale=invni_t[:rt, :])

            # rowmax tree + 2x duplication for the packed compare
            rmax = tree_reduce(S16, rt, OP.max, f"mx{ti % 2}")
            rdup = small.tile([128, 2 * BC], FP16, tag="rdup")
            nc.vector.tensor_copy(
                rdup[:rt, :].rearrange("p (c t) -> p c t", t=2),
                rmax[:rt, :, None].to_broadcast([rt, BC, 2]))

            # hard = (S16 == rowmax)   [2x_1p packed fp16 compare]
            nc.vector.tensor_tensor(
                vdup(hard, rt), vdup(S16, rt),
                rdup[:rt, :].rearrange("p (c t) -> p c t", t=2)[:, :, None, :]
                    .to_broadcast([rt, BC, W // 2, 2]),
                OP.is_equal)

            # den via tree (per-chunk exps already issued above)
            den = tree_reduce(expv, rt, OP.add, f"dn{ti % 2}")
            invden = small.tile([128, BC], F32, tag="invden")
            nc.vector.reciprocal(invden[:rt, :], den[:rt, :])
            idup = small.tile([128, 2 * BC], BF16, tag="idup")
            nc.vector.tensor_scalar_mul(
                idup[:rt, :].rearrange("p (c t) -> p c t", t=2),
                invden[:rt, :, None].to_broadcast([rt, BC, 2]), r_mix)

            # mixed = hard + (r/den)*exp, then *= nc*mask -> padded mixed
            nc.vector.tensor_tensor(
                vdup(expv, rt), vdup(expv, rt),
                idup[:rt, :].rearrange("p (c t) -> p c t", t=2)[:, :, None, :]
                    .to_broadcast([rt, BC, W // 2, 2]),
                OP.mult)
            nc.vector.tensor_tensor(vp(expv, rt), vp(expv, rt), vp(hard, rt),
                                    OP.add)
            nc.vector.tensor_tensor(vp(mp, rt), vp(expv, rt), vp(ncmask, rt),
                                    OP.mult)

            # num = sum_w mixed * S   (product into hard, tree reduce)
            nc.vector.tensor_tensor(vp(hard, rt), vp(mp, rt), vp(S16, rt),
                                    OP.mult)
            num = tree_reduce(hard, rt, OP.add, f"nm{ti % 2}")

            # all 32 pair-block transposes in one XBAR DMA
            M_T = work.tile([128, NP, 128], BF16, tag="MT")
            nc.sync.dma_start_transpose(out=M_T[:], in_=mp[:])
            state[ti] = dict(mp=mp, M_T=M_T, expv=expv, num=num)

        def tile_back(ti):
            r0, rt = ROW_TILES[ti]
            st = state.pop(ti)
            mp, M_T, expv = st["mp"], st["M_T"], st["expv"]
            # u = G * mixed per pair; 4 pairs per PSUM tile, evac to U
            U = work.tile([128, CWP], BF16, tag="U")
            def vg(t, g):     # group g's 8 caption-slots, 50-col views
                return t[:rt, 512 * g:512 * (g + 1)].rearrange(
                    "p (c w) -> p c w", w=WP)[:, :, 0:W]

            for g in range(NP // 4):
                ups = psU.tile([128, 512], F32, tag="u")
                for jj in range(4):
                    j = 4 * g + jj
                    nc.tensor.matmul(ups[:, 128 * jj:128 * (jj + 1)],
                                     M_T[:, j, :], Gp[:, j, :],
                                     start=True, stop=True)
                nc.scalar.activation(U[:rt, 512 * g:512 * (g + 1)],
                                     ups[:rt, :], AF.Copy)
                # qf product for this group right after its evac
                nc.vector.tensor_tensor(vg(expv, g), vg(mp, g), vg(U, g),
                                        OP.mult)
            qf = tree_reduce(expv, rt, OP.add, f"qf{ti % 2}")

            nc.scalar.dma_start(out=out_num[r0:r0 + rt, :],
                                in_=st["num"][:rt, :])
            nc.scalar.dma_start(out=out_qf[r0:r0 + rt, :], in_=qf[:rt, :])

        for ti in range(len(ROW_TILES) + 1):
            if ti < len(ROW_TILES):
                tile_front(ti)
            if ti >= 1:
                tile_back(ti - 1)

    nc.finalize()
    return nc


def _get_runner(r_mix: float):
    key = round(float(r_mix), 9)
    if key not in _CACHE:
        _CACHE[key] = _build(key)
    return _CACHE[key]


def _host_prep(imgs, caps, img_lens, cap_lens):
    imgs = np.ascontiguousarray(np.asarray(imgs, dtype=np.float32))
    caps = np.ascontiguousarray(np.asarray(caps, dtype=np.float32))
    cap_lens = np.asarray(cap_lens).astype(np.int64)

    ncn = np.linalg.norm(caps, axis=-1) + EPS            # [Bc, W]
    cn = caps / ncn[..., None]
    cnb = cn.astype(ml_dtypes.bfloat16).astype(np.float32)
    # caps packed so each (kc, half) DMA piece is contiguous in DRAM
    capsT = cn.reshape(CW, D).T                          # [D, CW]
    caps_pk = np.ascontiguousarray(
        capsT.reshape(KC, 128, 2, CW // 2).transpose(0, 2, 1, 3)).astype(
        ml_dtypes.bfloat16)                              # [KC, 2, 128, 1600]
    cap_mask = (np.arange(W)[None, :] < cap_lens[:, None])  # [Bc, W]
    adds_row = np.where(cap_mask.reshape(1, CW), 0.0,
                        NEGS).astype(ml_dtypes.bfloat16)
    ncm_row = np.where(cap_mask, ncn, 0.0).reshape(1, CW).astype(
        ml_dtypes.bfloat16)
    inv_ni = (10.0 / (np.linalg.norm(imgs, axis=-1) + EPS)).astype(
        np.float32)                                      # [B, R]
    # pair-block-diagonal normalized caption Gram, computed on host:
    # gp_in[p, j, m]: G_{2j} at [0:50, j, 0:50], G_{2j+1} at [64:114, j, 64:]
    G = np.einsum('cwd,cvd->cwv', cnb, cnb)              # [Bc, W, W]
    gp = np.zeros((128, NP, 128), np.float32)
    gp[:W, :, :W] = G[0::2].transpose(1, 0, 2)
    gp[WP:WP + W, :, WP:WP + W] = G[1::2].transpose(1, 0, 2)
    gp_in = gp.astype(ml_dtypes.bfloat16)

    in_maps = []
    for core in range(N_CORES):
        sl = slice(core * BI, (core + 1) * BI)
        imT = imgs[sl].reshape(ROWS, D).T                # [D, ROWS]
        imgs_pk = np.ascontiguousarray(
            imT.reshape(KC, 128, ROWS).transpose(1, 0, 2)).astype(
            ml_dtypes.bfloat16)                          # [128, KC, ROWS]
        in_maps.append({
            "caps_pk": caps_pk,
            "gp_in": gp_in,
            "imgs_pk": imgs_pk,
            "adds_row": adds_row,
            "ncm_row": ncm_row,
            "invni_col": np.ascontiguousarray(
                inv_ni[sl].reshape(ROWS, 1)),
        })
    return in_maps


def run_on_device(inputs: dict, trace: bool = False):
    """Returns (output [64,64,36] f32, BassKernelResults)."""
    from concourse.bass_utils import run_bass_kernel_spmd
    alpha = float(np.asarray(inputs["alpha"]).reshape(-1)[0])
    a = 1.0 / (1.0 + np.exp(-alpha))
    r_mix = a / max(1.0 - a, 1e-9)
    nc = _get_runner(r_mix)
    in_maps = _host_prep(inputs["imgs"], inputs["caps"], inputs["img_lens"],
                         inputs["cap_lens"])
    r = run_bass_kernel_spmd(nc, in_maps, list(range(N_CORES)), trace=trace)
    img_lens = np.asarray(inputs["img_lens"]).astype(np.int64)
    iv = (np.arange(R)[None, :] < img_lens[:, None])     # [B, R]
    outs = []
    for c in range(N_CORES):
        num = r.results[c]["out_num"].astype(np.float32)  # [ROWS, BC]
        qf = r.results[c]["out_qf"].astype(np.float32)
        o = num / (np.sqrt(np.maximum(qf, 0.0)) + 1e-30)
        o = o.reshape(BI, R, BC).transpose(0, 2, 1)       # [BI, BC, R]
        o = np.where(iv[c * BI:(c + 1) * BI, None, :], o, -1.0)
        outs.append(o)
    return np.concatenate(outs, axis=0).astype(np.float32), r


def kernel(imgs, caps, img_lens, cap_lens, alpha):
    out, _ = run_on_device({"imgs": imgs, "caps": caps, "img_lens": img_lens,
                            "cap_lens": cap_lens, "alpha": alpha})
    return out
